# revision 1
# baseline (speedup 1.0000x reference)
"""Trainium2 Bass kernel for nn_Block_4526895530469 (Mamba block + MLP residual).

Sharding over 8 NeuronCores: core c -> batch b=c//4, channel shard r=c%4
(512 of the 2048 d_inner channels), full T=2048 sequence per core. The
selective scan runs full-T per channel on the Vector engine
(tensor_tensor_scan), so there is no cross-core state chain. Layout for the
scan is (s,e)-pairs on partitions (row p = 8*s + e_local, 16 states x 8
channels per 128-row tile) x time on the free dimension.

Collectives: one zero-padded global AllReduce for the (96, T) x_dbl partial
sums (contraction over the sharded d_inner), and one 4-group ReduceScatter
for the out_proj partials which simultaneously scatters tokens for the
token-parallel MLP tail.
"""
import sys
sys.path.insert(0, '/opt/trn_rl_repo')

import numpy as np
from contextlib import ExitStack

import concourse.bass as bass
from concourse import bacc
import concourse.tile as tile
from concourse import mybir
from concourse.bass_utils import run_bass_kernel_spmd

# The interp (used by Tile's scheduling pass and by test simulation) lacks
# Silu; emulate it: run the existing Sigmoid path, then multiply by the
# scaled/biased input.
from concourse import bass_interp as _bi
from concourse import mybir as _mb

_orig_visit_act = _bi.InstructionExecutor.visit_InstActivation


def _visit_act_with_silu(self, instruction, *a, **kw):
    if instruction.func != _mb.ActivationFunctionType.Silu:
        return _orig_visit_act(self, instruction, *a, **kw)
    import numpy as _np
    assert len(instruction.outs) == 1, "Silu shim: no accum_out support"
    func0 = instruction.func
    try:
        instruction.func = _mb.ActivationFunctionType.Sigmoid
        res = _orig_visit_act(self, instruction, *a, **kw)
    finally:
        instruction.func = func0
    reg_snapshot = kw.get("reg_snapshot")
    inp = self.view_ap(instruction.ins[0], _bi.Direction.READ, instruction,
                       reg_snapshot=reg_snapshot).astype(_np.float32)
    inp = inp.reshape(inp.shape[0], -1)

    def _val(arg):
        if isinstance(arg, _mb.ImmediateValue):
            return arg.value
        v = self.view_ap(arg, _bi.Direction.READ, instruction,
                         reg_snapshot=reg_snapshot).astype(_np.float32)
        return v.reshape(v.shape[0], -1)

    bias = _val(instruction.ins[1])
    scale = _val(instruction.ins[2])
    sx = inp * scale + bias
    out_view = self.view_ap(instruction.outs[0], _bi.Direction.WRITE, instruction,
                            reg_snapshot=reg_snapshot)
    sig = _np.asarray(out_view, dtype=_np.float32).reshape(sx.shape)
    out_view[:] = (sig * sx).reshape(out_view.shape).astype(out_view.dtype)
    return res


_bi.InstructionExecutor.visit_InstActivation = _visit_act_with_silu

F32 = mybir.dt.float32
BF16 = mybir.dt.bfloat16
AF = mybir.ActivationFunctionType
ALU = mybir.AluOpType

D_MODEL, D_INNER, D_STATE, D_CONV, DT_RANK = 1024, 2048, 16, 4, 64
B, T = 2, 2048
EL = D_INNER // 4          # 512 channels per core
NET = EL // 128            # 4 e-tiles
NJ = EL // 8               # 64 scan tiles
NCH = T // 512             # 4 t-chunks
TQ = T // 4                # 512 tokens for the MLP tail
XD = DT_RANK + 2 * D_STATE  # 96
EPS = float(np.finfo(np.float32).eps)

_CACHE = {}


def _build(nocc=False, gps_mod=2, ar_bf16=True, nocc_ar=False, nocc_rs=False, rs_split=True):
    nc = bacc.Bacc("TRN2", target_bir_lowering=False, debug=False, num_devices=8)

    def din(name, shape, dt=BF16):
        return nc.dram_tensor(name, list(shape), dt, kind="ExternalInput").ap()

    xb = din("xb", (T, D_MODEL), F32)
    xq = din("xq", (TQ, D_MODEL), F32)
    w_in_T = din("w_in_T", (D_MODEL, 2 * EL))
    conv_wc = din("conv_wc", (128, NET * D_CONV), F32)   # cols [4k:4k+4] = e-tile k
    conv_bc = din("conv_bc", (128, NET), F32)
    w_xp_T = din("w_xp_T", (EL, XD))
    w_dt_T = din("w_dt_T", (DT_RANK, EL))
    dt_bnc = din("dt_bnc", (128, NET), F32)              # -dt_proj_b
    negA = din("negA", (128, NJ), F32)                   # exp(A_log) per scan tile col
    d_c = din("d_c", (128, NET), F32)
    w_out_T = din("w_out_T", (EL, D_MODEL))
    w_fc_T = din("w_fc_T", (D_MODEL, 2 * D_MODEL))
    w_pr_T = din("w_pr_T", (2 * D_MODEL, D_MODEL))
    r01 = din("r01", (16 * 128, 128))
    g01 = din("g01", (16 * 128, 128))
    s01n = din("s01n", (D_STATE, 128))
    s01p = din("s01p", (D_STATE, 128))
    ident_bf = din("ident_bf", (128, 128))
    ident_f32 = din("ident_f32", (128, 128), F32)
    ones_bf = din("ones_bf", (128, 1))
    ones_row_bf = din("ones_row_bf", (1, 128))
    msk0 = din("msk0", (XD, 1), F32)
    msk1 = din("msk1", (XD, 1), F32)

    out = nc.dram_tensor("out", [TQ, D_MODEL], F32, kind="ExternalOutput").ap()

    xdbl_in = nc.dram_tensor("xdbl_in", [2 * XD, T], BF16).ap()
    xdbl_out = nc.dram_tensor("xdbl_out", [2 * XD, T], BF16).ap()
    rs_in_h = [nc.dram_tensor(f"rs_in{h}", [4 * D_MODEL, TQ // 2], BF16).ap() for h in range(2)]
    rs_out_h = [nc.dram_tensor(f"rs_out{h}", [D_MODEL, TQ // 2], BF16).ap() for h in range(2)]
    dtu_dram = nc.dram_tensor("dtu_dram", [EL, T], BF16).ap()
    xqT_dram = nc.dram_tensor("xqT_dram", [D_MODEL, TQ], F32).ap()

    g8 = [[0, 1, 2, 3, 4, 5, 6, 7]]
    g4 = [[0, 1, 2, 3], [4, 5, 6, 7]]

    with tile.TileContext(nc) as tc, ExitStack() as top:
        cpool = top.enter_context(tc.tile_pool(name="consts", bufs=1))

        def cload(nm, name_ap, shape, dt=BF16):
            t = cpool.tile(list(shape), dt, tag=nm, name=nm)
            nc.sync.dma_start(t[:], name_ap)
            return t

        negA_t = cload("negA_t", negA, (128, NJ), F32)
        convw_t = cload("convw_t", conv_wc, (128, NET * D_CONV), F32)
        convb_t = cload("convb_t", conv_bc, (128, NET), F32)
        dtbn_t = cload("dtbn_t", dt_bnc, (128, NET), F32)
        dcol_t = cload("dcol_t", d_c, (128, NET), F32)
        s01n_t = cload("s01n_t", s01n, (D_STATE, 128))
        s01p_t = cload("s01p_t", s01p, (D_STATE, 128))
        idb_t = cload("idb_t", ident_bf, (128, 128))
        idf_t = cload("idf_t", ident_f32, (128, 128), F32)
        ones_t = cload("ones_t", ones_bf, (128, 1))
        onesr_t = cload("onesr_t", ones_row_bf, (1, 128))
        msk0_t = cload("msk0_t", msk0, (XD, 1), F32)
        msk1_t = cload("msk1_t", msk1, (XD, 1), F32)
        eps_t = cpool.tile([128, 1], F32)
        nc.vector.memset(eps_t[:], EPS)
        r01_t = cpool.tile([128, 16 * 128], BF16)
        for k in range(16):
            nc.sync.dma_start(r01_t[:, 128 * k:128 * (k + 1)], r01[128 * k:128 * (k + 1), :])
        g01_t = cpool.tile([128, 16 * 128], BF16)
        for k in range(16):
            nc.sync.dma_start(g01_t[:, 128 * k:128 * (k + 1)], g01[128 * k:128 * (k + 1), :])

        # long-lived activations through the scan phase (freed before MLP)
        mid = top.enter_context(ExitStack())
        acts = mid.enter_context(tc.tile_pool(name="acts", bufs=1))
        u_bf = [acts.tile([128, T], BF16, tag=f"u{k}", name=f"u{k}") for k in range(NET)]
        sz_bf = [acts.tile([128, T], BF16, tag=f"sz{k}", name=f"sz{k}") for k in range(NET)]
        lnsig_bf = [acts.tile([128, T], BF16, tag=f"lns{k}", name=f"lns{k}") for k in range(NET)]
        brep_t = acts.tile([128, T], BF16, tag="brep", name="brep")
        crep_t = acts.tile([128, T], BF16, tag="crep", name="crep")
        dtlow_bf = acts.tile([DT_RANK, T], BF16, tag="dtlow", name="dtlow")

        # ============ P1+P2: rmsnorm, transpose, in_proj ============
        with ExitStack() as ph:
            winp = ph.enter_context(tc.tile_pool(name="win", bufs=1))
            w_in_t = [winp.tile([128, 2 * EL], BF16, tag=f"wi{k}", name=f"wi{k}") for k in range(8)]
            for k in range(8):
                nc.sync.dma_start(w_in_t[k][:], w_in_T[128 * k:128 * (k + 1), :])

            xinp = ph.enter_context(tc.tile_pool(name="xinz", bufs=1))
            x_in = [xinp.tile([128, T], BF16, tag=f"xin{k}", name=f"xin{k}") for k in range(NET)]

            with ExitStack() as p1:
                xnp = p1.enter_context(tc.tile_pool(name="xnT", bufs=1))
                xn_T = [xnp.tile([128, T], BF16, tag=f"xnT{k}", name=f"xnT{k}") for k in range(8)]
                str_p = p1.enter_context(tc.tile_pool(name="p1s", bufs=3))
                xtmp = p1.enter_context(tc.tile_pool(name="p1x", bufs=6))
                jp = p1.enter_context(tc.tile_pool(name="p1j", bufs=1))
                junk = jp.tile([128, D_MODEL], BF16)
                pps1 = p1.enter_context(tc.tile_pool(name="p1ps", bufs=2, space="PSUM"))

                for c in range(NCH):
                    xnt4 = []
                    for q in range(4):
                        i = 4 * c + q
                        xt = str_p.tile([128, D_MODEL], BF16, tag="xt", name="xt")
                        nc.gpsimd.dma_start(xt[:], xb[128 * i:128 * (i + 1), :])
                        ssq = str_p.tile([128, 1], F32, tag="ssq", name="ssq")
                        nc.scalar.activation(junk[:], xt[:], AF.Square, accum_out=ssq[:])
                        rr = str_p.tile([128, 1], F32, tag="rr", name="rr")
                        nc.scalar.activation(rr[:], ssq[:], AF.Sqrt, scale=1.0 / D_MODEL,
                                             bias=eps_t[:, 0:1])
                        rc = str_p.tile([128, 1], F32, tag="rc", name="rc")
                        nc.vector.reciprocal(rc[:], rr[:])
                        xnt = xtmp.tile([128, D_MODEL], BF16, tag="xnt", name="xnt")
                        nc.vector.tensor_scalar_mul(xnt[:], xt[:], rc[:, 0:1])
                        xnt4.append(xnt)
                    for k in range(8):
                        pt = pps1.tile([128, 512], BF16, tag="pt", name="pt")
                        for q in range(4):
                            nc.tensor.transpose(pt[:, 128 * q:128 * (q + 1)],
                                                xnt4[q][:, 128 * k:128 * (k + 1)],
                                                idb_t[:])
                        nc.vector.tensor_copy(xn_T[k][:, 512 * c:512 * (c + 1)], pt[:])

                # in_proj
                pps2 = p1.enter_context(tc.tile_pool(name="p2ps", bufs=4, space="PSUM"))
                for m in range(8):
                    for c in range(NCH):
                        ps = pps2.tile([128, 512], F32, tag="ps", name="ps")
                        for k in range(8):
                            nc.tensor.matmul(ps[:], w_in_t[k][:, 128 * m:128 * (m + 1)],
                                             xn_T[k][:, 512 * c:512 * (c + 1)],
                                             start=(k == 0), stop=(k == 7))
                        if m < 4:
                            nc.scalar.copy(x_in[m][:, 512 * c:512 * (c + 1)], ps[:])
                        else:
                            nc.scalar.activation(sz_bf[m - 4][:, 512 * c:512 * (c + 1)],
                                                 ps[:], AF.Silu)

            # ============ P3: conv + silu -> u ============
            with ExitStack() as p3:
                c3 = p3.enter_context(tc.tile_pool(name="p3", bufs=2))
                for k in range(NET):
                    xc = c3.tile([128, T], BF16, tag="xc", name="xc")
                    nc.vector.tensor_scalar(xc[:], x_in[k][:], convw_t[:, 4 * k + 3:4 * k + 4],
                                            convb_t[:, k:k + 1], ALU.mult, ALU.add)
                    for sh in range(1, 4):
                        nc.vector.scalar_tensor_tensor(
                            xc[:, sh:T], x_in[k][:, 0:T - sh],
                            convw_t[:, 4 * k + 3 - sh:4 * k + 4 - sh],
                            xc[:, sh:T], ALU.mult, ALU.add)
                    nc.scalar.activation(u_bf[k][:], xc[:], AF.Silu)

        # ============ P4: x_proj partial -> AllReduce8 -> dtlow/B_rep/C_rep ============
        with ExitStack() as p4:
            wxp = p4.enter_context(tc.tile_pool(name="wxp", bufs=1))
            w_xp_t = [wxp.tile([128, XD], BF16, tag=f"wxp{k}", name=f"wxp{k}") for k in range(NET)]
            for k in range(NET):
                nc.sync.dma_start(w_xp_t[k][:], w_xp_T[128 * k:128 * (k + 1), :])
            pps = p4.enter_context(tc.tile_pool(name="p4ps", bufs=2, space="PSUM"))
            sp = p4.enter_context(tc.tile_pool(name="p4s", bufs=2))
            big = p4.enter_context(tc.tile_pool(name="p4big", bufs=1))
            for c in range(NCH):
                ps = pps.tile([XD, 512], F32, tag="ps4", name="ps4")
                for k in range(NET):
                    nc.tensor.matmul(ps[:], w_xp_t[k][:], u_bf[k][:, 512 * c:512 * (c + 1)],
                                     start=(k == 0), stop=(k == NET - 1))
                t0 = sp.tile([XD, 512], BF16, tag="t0", name="t0")
                nc.vector.tensor_scalar_mul(t0[:], ps[:], msk0_t[:, 0:1])
                t1 = sp.tile([XD, 512], BF16, tag="t1", name="t1")
                nc.vector.tensor_scalar_mul(t1[:], ps[:], msk1_t[:, 0:1])
                nc.sync.dma_start(xdbl_in[0:XD, 512 * c:512 * (c + 1)], t0[:])
                nc.sync.dma_start(xdbl_in[XD:2 * XD, 512 * c:512 * (c + 1)], t1[:])
            if nocc or nocc_ar:
                nc.sync.dma_start(xdbl_out, xdbl_in)
            else:
                nc.gpsimd.collective_compute("AllReduce", ALU.add, replica_groups=g8,
                                             ins=[xdbl_in], outs=[xdbl_out])
            # fill the AllReduce window: transpose the residual token-quarter
            # (f32) now and stage it in DRAM for the MLP tail
            ppq = p4.enter_context(tc.tile_pool(name="p4q", bufs=2, space="PSUM"))
            spq = p4.enter_context(tc.tile_pool(name="p4qs", bufs=2))
            for i in range(TQ // 128):
                xt_ = spq.tile([128, D_MODEL], F32, tag="xq_tm", name="xq_tm")
                nc.sync.dma_start(xt_[:], xq[128 * i:128 * (i + 1), :])
                for h in range(2):
                    ptq = ppq.tile([128, 512], F32, tag="ptq", name="ptq")
                    for q in range(4):
                        k = 4 * h + q
                        nc.tensor.transpose(ptq[:, 128 * q:128 * (q + 1)],
                                            xt_[:, 128 * k:128 * (k + 1)], idf_t[:])
                    otq = spq.tile([128, 512], F32, tag="otq", name="otq")
                    nc.scalar.copy(otq[:], ptq[:])
                    for q in range(4):
                        k = 4 * h + q
                        nc.sync.dma_start(
                            xqT_dram[128 * k:128 * (k + 1), 128 * i:128 * (i + 1)],
                            otq[:, 128 * q:128 * (q + 1)])
            # batch-select each 32-aligned section separately (compute ops
            # cannot start at partition 80)
            def _sel(rows, nrows, out_dt, nm):
                a0 = big.tile([nrows, T], BF16, tag=nm + "a0", name=nm + "a0")
                nc.sync.dma_start(a0[:], xdbl_out[rows:rows + nrows, :])
                a1 = big.tile([nrows, T], BF16, tag=nm + "a1", name=nm + "a1")
                nc.sync.dma_start(a1[:], xdbl_out[XD + rows:XD + rows + nrows, :])
                nc.vector.tensor_scalar_mul(a0[:], a0[:], msk0_t[0:nrows, 0:1])
                o = big.tile([nrows, T], out_dt, tag=nm, name=nm)
                nc.vector.scalar_tensor_tensor(o[:], a1[:], msk1_t[0:nrows, 0:1],
                                               a0[:], ALU.mult, ALU.add)
                return o
            dl = _sel(0, DT_RANK, BF16, "dl")
            nc.vector.tensor_copy(dtlow_bf[:], dl[:])
            b_sb = _sel(DT_RANK, D_STATE, BF16, "b_sb")
            c_sb = _sel(DT_RANK + D_STATE, D_STATE, BF16, "c_sb")
            pps2 = p4.enter_context(tc.tile_pool(name="p4ps2", bufs=2, space="PSUM"))
            for c in range(NCH):
                pb = pps2.tile([128, 512], F32, tag="pb", name="pb")
                nc.tensor.matmul(pb[:], s01n_t[:], b_sb[:, 512 * c:512 * (c + 1)],
                                 start=True, stop=True)
                nc.vector.tensor_copy(brep_t[:, 512 * c:512 * (c + 1)], pb[:])
                pc = pps2.tile([128, 512], F32, tag="pc", name="pc")
                nc.tensor.matmul(pc[:], s01p_t[:], c_sb[:, 512 * c:512 * (c + 1)],
                                 start=True, stop=True)
                nc.vector.tensor_copy(crep_t[:, 512 * c:512 * (c + 1)], pc[:])

        # ============ P5: dt path ============
        with ExitStack() as p5:
            wdt = p5.enter_context(tc.tile_pool(name="wdt", bufs=1))
            w_dt_t = wdt.tile([DT_RANK, EL], BF16)
            nc.sync.dma_start(w_dt_t[:], w_dt_T)
            pps = p5.enter_context(tc.tile_pool(name="p5ps", bufs=4, space="PSUM"))
            sp = p5.enter_context(tc.tile_pool(name="p5s", bufs=2))
            for m in range(NET):
                sg = sp.tile([128, T], F32, tag="sg", name="sg")
                for c in range(NCH):
                    ps = pps.tile([128, 512], F32, tag="ps5", name="ps5")
                    nc.tensor.matmul(ps[:], w_dt_t[:, 128 * m:128 * (m + 1)],
                                     dtlow_bf[:, 512 * c:512 * (c + 1)], start=True, stop=True)
                    nc.scalar.activation(sg[:, 512 * c:512 * (c + 1)], ps[:], AF.Sigmoid,
                                         scale=-1.0, bias=dtbn_t[:, m:m + 1])
                lns = sp.tile([128, T], F32, tag="lns", name="lns")
                nc.scalar.activation(lns[:], sg[:], AF.Ln)
                nc.vector.tensor_copy(lnsig_bf[m][:], lns[:])
                dtu = sp.tile([128, T], BF16, tag="dtu", name="dtu")
                nc.vector.tensor_tensor(dtu[:], lns[:], u_bf[m][:], ALU.mult)
                nc.sync.dma_start(dtu_dram[128 * m:128 * (m + 1), :], dtu[:])

        # ============ P6: scan + y-sum + gate ============
        y2p = mid.enter_context(tc.tile_pool(name="y2p", bufs=1))
        y2_bf = [y2p.tile([128, T], BF16, tag=f"y2{k}", name=f"y2{k}") for k in range(NET)]
        with ExitStack() as p6:
            reps = p6.enter_context(tc.tile_pool(name="reps", bufs=2, space="PSUM"))
            yps = p6.enter_context(tc.tile_pool(name="ypsum", bufs=1, space="PSUM"))
            sp = p6.enter_context(tc.tile_pool(name="p6s", bufs=3))
            for J in range(4):
                py = yps.tile([128, T], F32, tag="py", name="py")
                for jj in range(16):
                    j = 16 * J + jj
                    dA = sp.tile([128, T], F32, tag="dA", name="dA")
                    for hf in range(2):
                        pr = reps.tile([128, 1024], F32, tag="pr", name="pr")
                        for q in range(2):
                            c = 2 * hf + q
                            nc.tensor.matmul(pr[:, 512 * q:512 * (q + 1)],
                                             r01_t[:, 128 * jj:128 * (jj + 1)],
                                             lnsig_bf[J][:, 512 * c:512 * (c + 1)],
                                             start=True, stop=True)
                        nc.scalar.activation(dA[:, 1024 * hf:1024 * (hf + 1)], pr[:],
                                             AF.Exp, scale=negA_t[:, j:j + 1])
                    dtur = sp.tile([128, T], BF16, tag="dtur", name="dtur")
                    src = dtu_dram[128 * J + 8 * jj:128 * J + 8 * jj + 8, :]
                    nc.sync.dma_start(dtur[:], src.unsqueeze(0).broadcast_to([16, 8, T]))
                    # scan is DVE-only (walrus rejects it on Pool); the two
                    # elementwise multiplies can run on GpSimd to unload DVE
                    bb = sp.tile([128, T], BF16, tag="bb", name="bb")
                    nc.vector.tensor_tensor(bb[:], dtur[:], brep_t[:], ALU.mult)
                    hh = sp.tile([128, T], BF16, tag="hh", name="hh")
                    nc.vector.tensor_tensor_scan(hh[:], dA[:], bb[:], 0.0, ALU.mult, ALU.add)
                    # C-multiply is off the scan-to-scan chain: mostly GpSimd,
                    # 1/3 back on DVE so Pool doesn't become the phase bound
                    ve = (nc.gpsimd if (j % 4) else nc.vector) if gps_mod else nc.vector
                    t1 = sp.tile([128, T], BF16, tag="t1", name="t1")
                    ve.tensor_tensor(t1[:], hh[:], crep_t[:], ALU.mult)
                    for c in range(NCH):
                        nc.tensor.matmul(py[:, 512 * c:512 * (c + 1)],
                                         g01_t[:, 128 * jj:128 * (jj + 1)],
                                         t1[:, 512 * c:512 * (c + 1)],
                                         start=(jj == 0), stop=(jj == 15))
                for c in range(NCH):
                    yd = sp.tile([128, 512], F32, tag="yd", name="yd")
                    nc.vector.scalar_tensor_tensor(yd[:], u_bf[J][:, 512 * c:512 * (c + 1)],
                                                   dcol_t[:, J:J + 1],
                                                   py[:, 512 * c:512 * (c + 1)],
                                                   ALU.mult, ALU.add)
                    nc.vector.tensor_tensor(y2_bf[J][:, 512 * c:512 * (c + 1)], yd[:],
                                            sz_bf[J][:, 512 * c:512 * (c + 1)], ALU.mult)

        # ============ P7: out_proj partial -> ReduceScatter4 ============
        with ExitStack() as p7:
            wout = p7.enter_context(tc.tile_pool(name="wout", bufs=1))
            w_out_t = [wout.tile([128, D_MODEL], BF16, tag=f"wo{k}", name=f"wo{k}") for k in range(NET)]
            for k in range(NET):
                nc.sync.dma_start(w_out_t[k][:], w_out_T[128 * k:128 * (k + 1), :])
            pps = p7.enter_context(tc.tile_pool(name="p7ps", bufs=4, space="PSUM"))
            sp = p7.enter_context(tc.tile_pool(name="p7s", bufs=4))
            for m in range(8):
                for c in range(NCH):
                    ps = pps.tile([128, 512], F32, tag="ps7", name="ps7")
                    for k in range(NET):
                        nc.tensor.matmul(ps[:], w_out_t[k][:, 128 * m:128 * (m + 1)],
                                         y2_bf[k][:, 512 * c:512 * (c + 1)],
                                         start=(k == 0), stop=(k == NET - 1))
                    ob = sp.tile([128, 512], BF16, tag="ob", name="ob")
                    nc.scalar.copy(ob[:], ps[:])
                    rr = slice(D_MODEL * c + 128 * m, D_MODEL * c + 128 * (m + 1))
                    nc.sync.dma_start(rs_in_h[0][rr, :], ob[:, 0:TQ // 2])
                    nc.sync.dma_start(rs_in_h[1][rr, :], ob[:, TQ // 2:TQ])
            for h in range(2):
                if nocc or nocc_rs:
                    nc.sync.dma_start(rs_out_h[h], rs_in_h[h][0:D_MODEL, :])
                else:
                    nc.gpsimd.collective_compute("ReduceScatter", ALU.add,
                                                 replica_groups=g4,
                                                 ins=[rs_in_h[h]], outs=[rs_out_h[h]])

        mid.close()

        # ============ P8: MLP tail ============
        with ExitStack() as p8:
            wmlp = p8.enter_context(tc.tile_pool(name="wmlp", bufs=1))
            w_fc_t = [wmlp.tile([128, 2 * D_MODEL], BF16, tag=f"wf{k}", name=f"wf{k}") for k in range(8)]
            for k in range(8):
                nc.sync.dma_start(w_fc_t[k][:], w_fc_T[128 * k:128 * (k + 1), :])
            w_pr_t = [wmlp.tile([128, D_MODEL], BF16, tag=f"wp{k}", name=f"wp{k}") for k in range(16)]
            for k in range(16):
                nc.sync.dma_start(w_pr_t[k][:], w_pr_T[128 * k:128 * (k + 1), :])

            ar = p8.enter_context(tc.tile_pool(name="p8a", bufs=1))
            st = p8.enter_context(tc.tile_pool(name="p8t", bufs=2))
            ppt = p8.enter_context(tc.tile_pool(name="p8pt", bufs=2, space="PSUM"))
            ppm = p8.enter_context(tc.tile_pool(name="p8pm", bufs=2, space="PSUM"))
            pp1 = p8.enter_context(tc.tile_pool(name="p8p1", bufs=1, space="PSUM"))

            TH = TQ // 2
            for th in range(2):
                t0 = TH * th
                x2_T = [ar.tile([128, TH], F32, tag=f"x2T{k}", name=f"x2T{k}")
                        for k in range(8)]
                for k in range(8):
                    nc.sync.dma_start(x2_T[k][:], xqT_dram[128 * k:128 * (k + 1), t0:t0 + TH])
                rsb = [ar.tile([128, TH], BF16, tag=f"rsb{k}", name=f"rsb{k}") for k in range(8)]
                for k in range(8):
                    nc.sync.dma_start(rsb[k][:], rs_out_h[th][128 * k:128 * (k + 1), :])
                    nc.vector.tensor_tensor(x2_T[k][:], x2_T[k][:], rsb[k][:], ALU.add)

                # rmsnorm over features via ones-matmul
                sq = [ar.tile([128, TH], BF16, tag=f"sq{k}", name=f"sq{k}") for k in range(8)]
                for k in range(8):
                    nc.scalar.activation(sq[k][:], x2_T[k][:], AF.Square)
                pss = pp1.tile([1, TH], F32, tag="pss", name="pss")
                for k in range(8):
                    nc.tensor.matmul(pss[:], ones_t[:], sq[k][:], start=(k == 0), stop=(k == 7))
                rrow = st.tile([1, TH], F32, tag="rrow", name="rrow")
                nc.scalar.activation(rrow[:], pss[:], AF.Sqrt, scale=1.0 / D_MODEL,
                                     bias=eps_t[0:1, 0:1])
                rrec = st.tile([1, TH], F32, tag="rrec", name="rrec")
                nc.vector.reciprocal(rrec[:], rrow[:])
                rbf = st.tile([1, TH], BF16, tag="rbf", name="rbf")
                nc.vector.tensor_copy(rbf[:], rrec[:])
                pr2 = pp1.tile([128, TH], F32, tag="pr2", name="pr2")
                nc.tensor.matmul(pr2[:], onesr_t[:], rbf[:], start=True, stop=True)
                x2n = [ar.tile([128, TH], BF16, tag=f"x2n{k}", name=f"x2n{k}") for k in range(8)]
                for k in range(8):
                    nc.vector.tensor_tensor(x2n[k][:], x2_T[k][:], pr2[:], ALU.mult)

                # c_fc + relu^2
                hh_t = [ar.tile([128, TH], BF16, tag=f"hh{k}", name=f"hh{k}") for k in range(16)]
                for m in range(16):
                    pm = ppm.tile([128, TH], F32, tag="pmm", name="pmm")
                    for k in range(8):
                        nc.tensor.matmul(pm[:], w_fc_t[k][:, 128 * m:128 * (m + 1)], x2n[k][:],
                                         start=(k == 0), stop=(k == 7))
                    rl = st.tile([128, TH], BF16, tag="rl", name="rl")
                    nc.scalar.activation(rl[:], pm[:], AF.Relu)
                    nc.vector.tensor_tensor(hh_t[m][:], rl[:], rl[:], ALU.mult)
                # c_proj + residual
                fin = [ar.tile([128, TH], F32, tag=f"fin{k}", name=f"fin{k}") for k in range(8)]
                for m in range(8):
                    pm = ppm.tile([128, TH], F32, tag="pmm", name="pmm")
                    for k in range(16):
                        nc.tensor.matmul(pm[:], w_pr_t[k][:, 128 * m:128 * (m + 1)], hh_t[k][:],
                                         start=(k == 0), stop=(k == 15))
                    nc.vector.tensor_tensor(fin[m][:], x2_T[m][:], pm[:], ALU.add)
                # transpose to token-major + store
                for i in range(TH // 128):
                    for h in range(2):
                        pt = ppt.tile([128, 512], F32, tag="ptx", name="ptx")
                        for q in range(4):
                            m = 4 * h + q
                            nc.tensor.transpose(pt[:, 128 * q:128 * (q + 1)],
                                                fin[m][:, 128 * i:128 * (i + 1)], idf_t[:])
                        ot = st.tile([128, 512], F32, tag="ot", name="ot")
                        nc.scalar.copy(ot[:], pt[:])
                        nc.sync.dma_start(out[t0 + 128 * i:t0 + 128 * (i + 1),
                                              512 * h:512 * (h + 1)], ot[:])

    nc.compile()
    return nc


def _prep_inputs(inputs):
    x = np.asarray(inputs['x'], np.float32)
    in_proj_w = np.asarray(inputs['in_proj_w'], np.float32)
    conv_w = np.asarray(inputs['conv_w'], np.float32)
    conv_b = np.asarray(inputs['conv_b'], np.float32)
    x_proj_w = np.asarray(inputs['x_proj_w'], np.float32)
    dt_proj_w = np.asarray(inputs['dt_proj_w'], np.float32)
    dt_proj_b = np.asarray(inputs['dt_proj_b'], np.float32)
    A_log = np.asarray(inputs['A_log'], np.float32)
    D = np.asarray(inputs['D'], np.float32)
    out_proj_w = np.asarray(inputs['out_proj_w'], np.float32)
    c_fc_w = np.asarray(inputs['c_fc_w'], np.float32)
    c_proj_w = np.asarray(inputs['c_proj_w'], np.float32)

    import ml_dtypes
    bf = lambda a: np.ascontiguousarray(a).astype(ml_dtypes.bfloat16)
    f32 = lambda a: np.ascontiguousarray(a, np.float32)

    r01 = np.zeros((16, 128, 128), np.float32)  # [jm][k, m] = 1 iff k == 8*jm + m%8
    g01 = np.zeros((16, 128, 128), np.float32)  # [jm][k, m] = 1 iff m == 8*jm + k%8
    for jm in range(16):
        for m in range(128):
            r01[jm, 8 * jm + (m % 8), m] = 1.0
            g01[jm, m, 8 * jm + (m % 8)] = 1.0
    s01n = np.zeros((D_STATE, 128), np.float32)
    s01p = np.zeros((D_STATE, 128), np.float32)
    for m in range(128):
        s01n[m // 8, m] = -1.0
        s01p[m // 8, m] = 1.0
    ident = np.eye(128, dtype=np.float32)

    def col_fold(a):
        # (EL,) or (EL, w) -> (128, NET*w): cols [w*k:w*(k+1)] = rows of e-tile k
        a = a.reshape(EL, -1)
        w = a.shape[1]
        o = np.zeros((128, NET * w), np.float32)
        for k in range(NET):
            o[:, w * k:w * (k + 1)] = a[128 * k:128 * (k + 1)]
        return o

    in_maps = []
    for c in range(8):
        b, r = c // 4, c % 4
        sl = slice(EL * r, EL * (r + 1))
        negA_ = np.zeros((128, NJ), np.float32)
        p = np.arange(128)
        for j in range(NJ):
            e = EL * r + 8 * j + (p % 8)
            s = p // 8
            negA_[:, j] = np.exp(A_log[e, s])
        msk0 = np.full((XD, 1), 1.0 if b == 0 else 0.0, np.float32)
        msk1 = np.full((XD, 1), 1.0 if b == 1 else 0.0, np.float32)
        in_maps.append({
            'xb': f32(x[b]),
            'xq': f32(x[b][TQ * r:TQ * (r + 1)]),
            'w_in_T': bf(np.concatenate([in_proj_w[sl], in_proj_w[D_INNER:][sl]], 0).T),
            'conv_wc': col_fold(conv_w[sl]),
            'conv_bc': col_fold(conv_b[sl]),
            'w_xp_T': bf(x_proj_w[:, sl].T),
            'w_dt_T': bf(dt_proj_w[sl].T),
            'dt_bnc': col_fold(-dt_proj_b[sl]),
            'negA': negA_,
            'd_c': col_fold(D[sl]),
            'w_out_T': bf(out_proj_w[:, sl].T),
            'w_fc_T': bf(c_fc_w.T),
            'w_pr_T': bf(c_proj_w.T),
            'r01': bf(r01.reshape(16 * 128, 128)),
            'g01': bf(g01.reshape(16 * 128, 128)),
            's01n': bf(s01n),
            's01p': bf(s01p),
            'ident_bf': bf(ident),
            'ident_f32': f32(ident),
            'ones_bf': bf(np.ones((128, 1), np.float32)),
            'ones_row_bf': bf(np.ones((1, 128), np.float32)),
            'msk0': msk0,
            'msk1': msk1,
        })
    return in_maps


def kernel(**inputs) -> np.ndarray:
    if 'nc' not in _CACHE:
        _CACHE['nc'] = _build()
    nc = _CACHE['nc']
    in_maps = _prep_inputs(inputs)
    res = run_bass_kernel_spmd(nc, in_maps, core_ids=list(range(8)))
    out = np.zeros((B, T, D_MODEL), np.float32)
    for c in range(8):
        b, r = c // 4, c % 4
        out[b, TQ * r:TQ * (r + 1), :] = res.results[c]['out']
    return out



# revision 24
# speedup vs baseline: 1.0416x; 1.0416x over previous
"""Trainium2 Bass kernel for nn_Block_4526895530469 (Mamba block + MLP residual).

Sharding over 8 NeuronCores: core c -> batch b=c//4, channel shard r=c%4
(512 of the 2048 d_inner channels), full T=2048 sequence per core. The
selective scan runs full-T per channel on the Vector engine
(tensor_tensor_scan), so there is no cross-core state chain. Layout for the
scan is (s,e)-pairs on partitions (row p = 8*s + e_local, 16 states x 8
channels per 128-row tile) x time on the free dimension.

Collectives: one zero-padded global AllReduce for the (96, T) x_dbl partial
sums (contraction over the sharded d_inner), and one 4-group ReduceScatter
for the out_proj partials which simultaneously scatters tokens for the
token-parallel MLP tail.
"""
import sys
sys.path.insert(0, '/opt/trn_rl_repo')

import numpy as np
from contextlib import ExitStack

import concourse.bass as bass
from concourse import bacc
import concourse.tile as tile
from concourse import mybir
from concourse.bass_utils import run_bass_kernel_spmd

# The interp (used by Tile's scheduling pass and by test simulation) lacks
# Silu; emulate it: run the existing Sigmoid path, then multiply by the
# scaled/biased input.
from concourse import bass_interp as _bi
from concourse import mybir as _mb

_orig_visit_act = _bi.InstructionExecutor.visit_InstActivation


def _visit_act_with_silu(self, instruction, *a, **kw):
    if instruction.func != _mb.ActivationFunctionType.Silu:
        return _orig_visit_act(self, instruction, *a, **kw)
    import numpy as _np
    assert len(instruction.outs) == 1, "Silu shim: no accum_out support"
    func0 = instruction.func
    try:
        instruction.func = _mb.ActivationFunctionType.Sigmoid
        res = _orig_visit_act(self, instruction, *a, **kw)
    finally:
        instruction.func = func0
    reg_snapshot = kw.get("reg_snapshot")
    inp = self.view_ap(instruction.ins[0], _bi.Direction.READ, instruction,
                       reg_snapshot=reg_snapshot).astype(_np.float32)
    inp = inp.reshape(inp.shape[0], -1)

    def _val(arg):
        if isinstance(arg, _mb.ImmediateValue):
            return arg.value
        v = self.view_ap(arg, _bi.Direction.READ, instruction,
                         reg_snapshot=reg_snapshot).astype(_np.float32)
        return v.reshape(v.shape[0], -1)

    bias = _val(instruction.ins[1])
    scale = _val(instruction.ins[2])
    sx = inp * scale + bias
    out_view = self.view_ap(instruction.outs[0], _bi.Direction.WRITE, instruction,
                            reg_snapshot=reg_snapshot)
    sig = _np.asarray(out_view, dtype=_np.float32).reshape(sx.shape)
    out_view[:] = (sig * sx).reshape(out_view.shape).astype(out_view.dtype)
    return res


_bi.InstructionExecutor.visit_InstActivation = _visit_act_with_silu

F32 = mybir.dt.float32
BF16 = mybir.dt.bfloat16
AF = mybir.ActivationFunctionType
ALU = mybir.AluOpType

D_MODEL, D_INNER, D_STATE, D_CONV, DT_RANK = 1024, 2048, 16, 4, 64
B, T = 2, 2048
EL = D_INNER // 4          # 512 channels per core
NET = EL // 128            # 4 e-tiles
NJ = EL // 8               # 64 scan tiles
NCH = T // 512             # 4 t-chunks
TQ = T // 4                # 512 tokens for the MLP tail
XD = DT_RANK + 2 * D_STATE  # 96
EPS = float(np.finfo(np.float32).eps)

_CACHE = {}


class _SkipBlock(Exception):
    pass


from contextlib import contextmanager


@contextmanager
def _skippable():
    try:
        yield
    except _SkipBlock:
        pass


def _build(nocc=False, gps_mod=2, ar_bf16=True, nocc_ar=False, nocc_rs=False, rs_split=True,
           ab_no_p6=False, ab_scan_tt=False, ab_no_dtur=False, ab_no_p1p2=False,
           ab_no_mlp=False, bb_pool_mod=2):
    nc = bacc.Bacc("TRN2", target_bir_lowering=False, debug=False, num_devices=8)

    def din(name, shape, dt=BF16):
        return nc.dram_tensor(name, list(shape), dt, kind="ExternalInput").ap()

    xb = din("xb", (T, D_MODEL), F32)
    xq = din("xq", (TQ, D_MODEL), F32)
    w_in_T = din("w_in_T", (D_MODEL, 2 * EL))
    conv_wc = din("conv_wc", (128, NET * D_CONV), F32)   # cols [4k:4k+4] = e-tile k
    conv_d = din("conv_d", (128, NET * D_CONV * 128))    # [128,128] diag blocks
    conv_bc = din("conv_bc", (128, NET), F32)
    w_xp_T = din("w_xp_T", (EL, XD))
    w_dt_T = din("w_dt_T", (DT_RANK, EL))
    dt_bnc = din("dt_bnc", (128, NET), F32)              # -dt_proj_b
    negA = din("negA", (128, NJ), F32)                   # exp(A_log) per scan tile col
    d_c = din("d_c", (128, NET), F32)
    w_out_T = din("w_out_T", (EL, D_MODEL))
    w_fc_T = din("w_fc_T", (D_MODEL, 2 * D_MODEL))
    w_pr_T = din("w_pr_T", (2 * D_MODEL, D_MODEL))
    r01 = din("r01", (16 * 128, 128))
    g01 = din("g01", (16 * 128, 128))
    s01n = din("s01n", (D_STATE, 128))
    s01p = din("s01p", (D_STATE, 128))
    ident_bf = din("ident_bf", (128, 128))
    ident_f32 = din("ident_f32", (128, 128), F32)
    ones_bf = din("ones_bf", (128, 1))
    ones_row_bf = din("ones_row_bf", (1, 128))
    msk0 = din("msk0", (XD, 1), F32)
    msk1 = din("msk1", (XD, 1), F32)

    out = nc.dram_tensor("out", [TQ, D_MODEL], F32, kind="ExternalOutput").ap()

    xdbl_in = nc.dram_tensor("xdbl_in", [2 * XD, T], BF16).ap()
    xdbl_out = nc.dram_tensor("xdbl_out", [2 * XD, T], BF16).ap()
    rs_in_h = [nc.dram_tensor(f"rs_in{h}", [4 * D_MODEL, TQ // 2], BF16).ap() for h in range(2)]
    rs_out_h = [nc.dram_tensor(f"rs_out{h}", [D_MODEL, TQ // 2], BF16).ap() for h in range(2)]
    dtu_dram = nc.dram_tensor("dtu_dram", [EL, T], BF16).ap()
    xqT_dram = nc.dram_tensor("xqT_dram", [D_MODEL, TQ], F32).ap()

    g8 = [[0, 1, 2, 3, 4, 5, 6, 7]]
    g4 = [[0, 1, 2, 3], [4, 5, 6, 7]]

    with tile.TileContext(nc) as tc, ExitStack() as top:
        cpool = top.enter_context(tc.tile_pool(name="consts", bufs=1))

        def cload(nm, name_ap, shape, dt=BF16):
            t = cpool.tile(list(shape), dt, tag=nm, name=nm)
            nc.sync.dma_start(t[:], name_ap)
            return t

        negA_t = cload("negA_t", negA, (128, NJ), F32)
        convd_t = cload("convd_t", conv_d, (128, NET * D_CONV * 128))
        convb_t = cload("convb_t", conv_bc, (128, NET), F32)
        dtbn_t = cload("dtbn_t", dt_bnc, (128, NET), F32)
        dcol_t = cload("dcol_t", d_c, (128, NET), F32)
        s01n_t = cload("s01n_t", s01n, (D_STATE, 128))
        s01p_t = cload("s01p_t", s01p, (D_STATE, 128))
        idb_t = cload("idb_t", ident_bf, (128, 128))
        idf_t = cload("idf_t", ident_f32, (128, 128), F32)
        ones_t = cload("ones_t", ones_bf, (128, 1))
        onesr_t = cload("onesr_t", ones_row_bf, (1, 128))
        msk0_t = cload("msk0_t", msk0, (XD, 1), F32)
        msk1_t = cload("msk1_t", msk1, (XD, 1), F32)
        eps_t = cpool.tile([128, 1], F32)
        nc.vector.memset(eps_t[:], EPS)
        r01_t = cpool.tile([128, 16 * 128], BF16)
        for k in range(16):
            nc.sync.dma_start(r01_t[:, 128 * k:128 * (k + 1)], r01[128 * k:128 * (k + 1), :])
        g01_t = cpool.tile([128, 16 * 128], BF16)
        for k in range(16):
            nc.sync.dma_start(g01_t[:, 128 * k:128 * (k + 1)], g01[128 * k:128 * (k + 1), :])

        # long-lived activations through the scan phase (freed before MLP)
        mid = top.enter_context(ExitStack())
        acts = mid.enter_context(tc.tile_pool(name="acts", bufs=1))
        u_bf = [acts.tile([128, T], BF16, tag=f"u{k}", name=f"u{k}") for k in range(NET)]
        sz_bf = [acts.tile([128, T], BF16, tag=f"sz{k}", name=f"sz{k}") for k in range(NET)]
        lnsig_bf = [acts.tile([128, T], BF16, tag=f"lns{k}", name=f"lns{k}") for k in range(NET)]
        brep_t = acts.tile([128, T], BF16, tag="brep", name="brep")
        crep_t = acts.tile([128, T], BF16, tag="crep", name="crep")
        dtlow_bf = acts.tile([DT_RANK, T], BF16, tag="dtlow", name="dtlow")

        # ============ P1+P2: rmsnorm, transpose, in_proj ============
        with ExitStack() as ph:
            winp = ph.enter_context(tc.tile_pool(name="win", bufs=1))
            w_in_t = [winp.tile([128, 2 * EL], BF16, tag=f"wi{k}", name=f"wi{k}") for k in range(8)]
            for k in range(8):
                nc.sync.dma_start(w_in_t[k][:], w_in_T[128 * k:128 * (k + 1), :])

            xinp = ph.enter_context(tc.tile_pool(name="xinz", bufs=1))
            # 3 zero pad columns in front so the causal-conv PE matmuls can
            # read shifted windows without bounds checks
            x_in = [xinp.tile([128, 3 + T], BF16, tag=f"xin{k}", name=f"xin{k}")
                    for k in range(NET)]
            for k in range(NET):
                nc.vector.memset(x_in[k][:, 0:3], 0.0)

            if ab_no_p1p2:
                for k in range(NET):
                    nc.vector.memset(x_in[k][:], 0.01)
                    nc.vector.memset(sz_bf[k][:], 0.01)
            with _skippable(), ExitStack() as p1:
                if ab_no_p1p2:
                    raise _SkipBlock
                xnp = p1.enter_context(tc.tile_pool(name="xnT", bufs=1))
                xn_T = [xnp.tile([128, T], BF16, tag=f"xnT{k}", name=f"xnT{k}") for k in range(8)]
                str_p = p1.enter_context(tc.tile_pool(name="p1s", bufs=3))
                xtmp = p1.enter_context(tc.tile_pool(name="p1x", bufs=6))
                jp = p1.enter_context(tc.tile_pool(name="p1j", bufs=1))
                junk = jp.tile([128, D_MODEL], BF16)
                pps1 = p1.enter_context(tc.tile_pool(name="p1ps", bufs=2, space="PSUM"))

                for c in range(NCH):
                    xnt4 = []
                    for q in range(4):
                        i = 4 * c + q
                        xt = str_p.tile([128, D_MODEL], BF16, tag="xt", name="xt")
                        nc.gpsimd.dma_start(xt[:], xb[128 * i:128 * (i + 1), :])
                        ssq = str_p.tile([128, 1], F32, tag="ssq", name="ssq")
                        nc.scalar.activation(junk[:], xt[:], AF.Square, accum_out=ssq[:])
                        rr = str_p.tile([128, 1], F32, tag="rr", name="rr")
                        nc.scalar.activation(rr[:], ssq[:], AF.Sqrt, scale=1.0 / D_MODEL,
                                             bias=eps_t[:, 0:1])
                        rc = str_p.tile([128, 1], F32, tag="rc", name="rc")
                        nc.vector.reciprocal(rc[:], rr[:])
                        xnt = xtmp.tile([128, D_MODEL], BF16, tag="xnt", name="xnt")
                        nc.vector.tensor_scalar_mul(xnt[:], xt[:], rc[:, 0:1])
                        xnt4.append(xnt)
                    for k in range(8):
                        pt = pps1.tile([128, 512], BF16, tag="pt", name="pt")
                        for q in range(4):
                            nc.tensor.transpose(pt[:, 128 * q:128 * (q + 1)],
                                                xnt4[q][:, 128 * k:128 * (k + 1)],
                                                idb_t[:])
                        nc.vector.tensor_copy(xn_T[k][:, 512 * c:512 * (c + 1)], pt[:])

                # in_proj
                pps2 = p1.enter_context(tc.tile_pool(name="p2ps", bufs=4, space="PSUM"))
                for m in range(8):
                    for c in range(NCH):
                        ps = pps2.tile([128, 512], F32, tag="ps", name="ps")
                        for k in range(8):
                            nc.tensor.matmul(ps[:], w_in_t[k][:, 128 * m:128 * (m + 1)],
                                             xn_T[k][:, 512 * c:512 * (c + 1)],
                                             start=(k == 0), stop=(k == 7))
                        if m < 4:
                            nc.scalar.copy(x_in[m][:, 3 + 512 * c:3 + 512 * (c + 1)], ps[:])
                        else:
                            nc.scalar.activation(sz_bf[m - 4][:, 512 * c:512 * (c + 1)],
                                                 ps[:], AF.Silu)

            # ============ P3: conv (PE diag-matmuls) + silu -> u ============
            with ExitStack() as p3:
                pps3 = p3.enter_context(tc.tile_pool(name="p3ps", bufs=2, space="PSUM"))
                for k in range(NET):
                    for c in range(NCH):
                        ps = pps3.tile([128, 512], F32, tag="psc", name="psc")
                        for tap in range(D_CONV):
                            sh = D_CONV - 1 - tap       # time shift for this tap
                            dcol = 128 * (D_CONV * k + tap)
                            nc.tensor.matmul(ps[:], convd_t[:, dcol:dcol + 128],
                                             x_in[k][:, 3 + 512 * c - sh:3 + 512 * (c + 1) - sh],
                                             start=(tap == 0), stop=(tap == D_CONV - 1))
                        nc.scalar.activation(u_bf[k][:, 512 * c:512 * (c + 1)], ps[:],
                                             AF.Silu, bias=convb_t[:, k:k + 1])

        # ============ P4: x_proj partial -> AllReduce8 -> dtlow/B_rep/C_rep ============
        with ExitStack() as p4:
            wxp = p4.enter_context(tc.tile_pool(name="wxp", bufs=1))
            w_xp_t = [wxp.tile([128, XD], BF16, tag=f"wxp{k}", name=f"wxp{k}") for k in range(NET)]
            for k in range(NET):
                nc.sync.dma_start(w_xp_t[k][:], w_xp_T[128 * k:128 * (k + 1), :])
            pps = p4.enter_context(tc.tile_pool(name="p4ps", bufs=2, space="PSUM"))
            sp = p4.enter_context(tc.tile_pool(name="p4s", bufs=2))
            big = p4.enter_context(tc.tile_pool(name="p4big", bufs=1))
            for c in range(NCH):
                ps = pps.tile([XD, 512], F32, tag="ps4", name="ps4")
                for k in range(NET):
                    nc.tensor.matmul(ps[:], w_xp_t[k][:], u_bf[k][:, 512 * c:512 * (c + 1)],
                                     start=(k == 0), stop=(k == NET - 1))
                t0 = sp.tile([XD, 512], BF16, tag="t0", name="t0")
                nc.vector.tensor_scalar_mul(t0[:], ps[:], msk0_t[:, 0:1])
                t1 = sp.tile([XD, 512], BF16, tag="t1", name="t1")
                nc.vector.tensor_scalar_mul(t1[:], ps[:], msk1_t[:, 0:1])
                nc.sync.dma_start(xdbl_in[0:XD, 512 * c:512 * (c + 1)], t0[:])
                nc.sync.dma_start(xdbl_in[XD:2 * XD, 512 * c:512 * (c + 1)], t1[:])
            if nocc or nocc_ar:
                nc.sync.dma_start(xdbl_out, xdbl_in)
            else:
                nc.gpsimd.collective_compute("AllReduce", ALU.add, replica_groups=g8,
                                             ins=[xdbl_in], outs=[xdbl_out])
            # fill the AllReduce window: transpose the residual token-quarter
            # (f32) now and stage it in DRAM for the MLP tail
            ppq = p4.enter_context(tc.tile_pool(name="p4q", bufs=2, space="PSUM"))
            spq = p4.enter_context(tc.tile_pool(name="p4qs", bufs=2))
            for i in range(TQ // 128):
                xt_ = spq.tile([128, D_MODEL], F32, tag="xq_tm", name="xq_tm")
                nc.sync.dma_start(xt_[:], xq[128 * i:128 * (i + 1), :])
                for h in range(2):
                    ptq = ppq.tile([128, 512], F32, tag="ptq", name="ptq")
                    for q in range(4):
                        k = 4 * h + q
                        nc.tensor.transpose(ptq[:, 128 * q:128 * (q + 1)],
                                            xt_[:, 128 * k:128 * (k + 1)], idf_t[:])
                    otq = spq.tile([128, 512], F32, tag="otq", name="otq")
                    nc.scalar.copy(otq[:], ptq[:])
                    for q in range(4):
                        k = 4 * h + q
                        nc.sync.dma_start(
                            xqT_dram[128 * k:128 * (k + 1), 128 * i:128 * (i + 1)],
                            otq[:, 128 * q:128 * (q + 1)])
            # batch-select each 32-aligned section separately (compute ops
            # cannot start at partition 80)
            def _sel(rows, nrows, out_dt, nm):
                a0 = big.tile([nrows, T], BF16, tag=nm + "a0", name=nm + "a0")
                nc.sync.dma_start(a0[:], xdbl_out[rows:rows + nrows, :])
                a1 = big.tile([nrows, T], BF16, tag=nm + "a1", name=nm + "a1")
                nc.sync.dma_start(a1[:], xdbl_out[XD + rows:XD + rows + nrows, :])
                nc.vector.tensor_scalar_mul(a0[:], a0[:], msk0_t[0:nrows, 0:1])
                o = big.tile([nrows, T], out_dt, tag=nm, name=nm)
                nc.vector.scalar_tensor_tensor(o[:], a1[:], msk1_t[0:nrows, 0:1],
                                               a0[:], ALU.mult, ALU.add)
                return o
            dl = _sel(0, DT_RANK, BF16, "dl")
            nc.vector.tensor_copy(dtlow_bf[:], dl[:])
            b_sb = _sel(DT_RANK, D_STATE, BF16, "b_sb")
            c_sb = _sel(DT_RANK + D_STATE, D_STATE, BF16, "c_sb")
            pps2 = p4.enter_context(tc.tile_pool(name="p4ps2", bufs=2, space="PSUM"))
            for c in range(NCH):
                pb = pps2.tile([128, 512], F32, tag="pb", name="pb")
                nc.tensor.matmul(pb[:], s01n_t[:], b_sb[:, 512 * c:512 * (c + 1)],
                                 start=True, stop=True)
                nc.vector.tensor_copy(brep_t[:, 512 * c:512 * (c + 1)], pb[:])
                pc = pps2.tile([128, 512], F32, tag="pc", name="pc")
                nc.tensor.matmul(pc[:], s01p_t[:], c_sb[:, 512 * c:512 * (c + 1)],
                                 start=True, stop=True)
                nc.vector.tensor_copy(crep_t[:, 512 * c:512 * (c + 1)], pc[:])

        # ============ P5: dt path ============
        with ExitStack() as p5:
            wdt = p5.enter_context(tc.tile_pool(name="wdt", bufs=1))
            w_dt_t = wdt.tile([DT_RANK, EL], BF16)
            nc.sync.dma_start(w_dt_t[:], w_dt_T)
            pps = p5.enter_context(tc.tile_pool(name="p5ps", bufs=4, space="PSUM"))
            sp = p5.enter_context(tc.tile_pool(name="p5s", bufs=2))
            for m in range(NET):
                sg = sp.tile([128, T], F32, tag="sg", name="sg")
                for c in range(NCH):
                    ps = pps.tile([128, 512], F32, tag="ps5", name="ps5")
                    nc.tensor.matmul(ps[:], w_dt_t[:, 128 * m:128 * (m + 1)],
                                     dtlow_bf[:, 512 * c:512 * (c + 1)], start=True, stop=True)
                    nc.scalar.activation(sg[:, 512 * c:512 * (c + 1)], ps[:], AF.Sigmoid,
                                         scale=-1.0, bias=dtbn_t[:, m:m + 1])
                lns = sp.tile([128, T], F32, tag="lns", name="lns")
                nc.scalar.activation(lns[:], sg[:], AF.Ln)
                nc.vector.tensor_copy(lnsig_bf[m][:], lns[:])
                dtu = sp.tile([128, T], BF16, tag="dtu", name="dtu")
                nc.vector.tensor_tensor(dtu[:], lns[:], u_bf[m][:], ALU.mult)
                nc.sync.dma_start(dtu_dram[128 * m:128 * (m + 1), :], dtu[:])

        # ============ P6: scan + y-sum + gate ============
        y2p = mid.enter_context(tc.tile_pool(name="y2p", bufs=1))
        y2_bf = [y2p.tile([128, T], BF16, tag=f"y2{k}", name=f"y2{k}") for k in range(NET)]
        if ab_no_p6:
            for k in range(NET):
                nc.vector.memset(y2_bf[k][:], 0.01)
        with _skippable(), ExitStack() as p6:
            if ab_no_p6:
                raise _SkipBlock
            reps = p6.enter_context(tc.tile_pool(name="reps", bufs=2, space="PSUM"))
            yps = p6.enter_context(tc.tile_pool(name="ypsum", bufs=1, space="PSUM"))
            sp = p6.enter_context(tc.tile_pool(name="p6s", bufs=3))
            for J in range(4):
                py = yps.tile([128, T], F32, tag="py", name="py")
                for jj in range(16):
                    j = 16 * J + jj
                    dA = sp.tile([128, T], F32, tag="dA", name="dA")
                    for hf in range(2):
                        pr = reps.tile([128, 1024], F32, tag="pr", name="pr")
                        for q in range(2):
                            c = 2 * hf + q
                            nc.tensor.matmul(pr[:, 512 * q:512 * (q + 1)],
                                             r01_t[:, 128 * jj:128 * (jj + 1)],
                                             lnsig_bf[J][:, 512 * c:512 * (c + 1)],
                                             start=True, stop=True)
                        nc.scalar.activation(dA[:, 1024 * hf:1024 * (hf + 1)], pr[:],
                                             AF.Exp, scale=negA_t[:, j:j + 1])
                    dtur = sp.tile([128, T], BF16, tag="dtur", name="dtur")
                    if ab_no_dtur:
                        nc.vector.tensor_copy(dtur[:], u_bf[J][:])
                    else:
                        src = dtu_dram[128 * J + 8 * jj:128 * J + 8 * jj + 8, :]
                        nc.sync.dma_start(dtur[:], src.unsqueeze(0).broadcast_to([16, 8, T]))
                    # scan is DVE-only (walrus rejects it on Pool); the two
                    # elementwise multiplies can run on GpSimd to unload DVE
                    bb = sp.tile([128, T], BF16, tag="bb", name="bb")
                    # balance the two flexible multiplies: DVE is the scan-bound
                    # engine (2 cyc/el for scans), Pool absorbs ~1.25 of the 2
                    # elementwise passes
                    vb = nc.gpsimd if (bb_pool_mod and j % bb_pool_mod) else nc.vector
                    vb.tensor_tensor(bb[:], dtur[:], brep_t[:], ALU.mult)
                    hh = sp.tile([128, T], BF16, tag="hh", name="hh")
                    if ab_scan_tt:
                        nc.vector.tensor_tensor(hh[:], dA[:], bb[:], ALU.mult)
                    else:
                        nc.vector.tensor_tensor_scan(hh[:], dA[:], bb[:], 0.0, ALU.mult, ALU.add)
                    # C-multiply is off the scan-to-scan chain: mostly GpSimd,
                    # 1/3 back on DVE so Pool doesn't become the phase bound
                    ve = (nc.gpsimd if (j % 4) else nc.vector) if gps_mod else nc.vector
                    t1 = sp.tile([128, T], BF16, tag="t1", name="t1")
                    ve.tensor_tensor(t1[:], hh[:], crep_t[:], ALU.mult)
                    for c in range(NCH):
                        nc.tensor.matmul(py[:, 512 * c:512 * (c + 1)],
                                         g01_t[:, 128 * jj:128 * (jj + 1)],
                                         t1[:, 512 * c:512 * (c + 1)],
                                         start=(jj == 0), stop=(jj == 15))
                for c in range(NCH):
                    yd = sp.tile([128, 512], F32, tag="yd", name="yd")
                    nc.vector.scalar_tensor_tensor(yd[:], u_bf[J][:, 512 * c:512 * (c + 1)],
                                                   dcol_t[:, J:J + 1],
                                                   py[:, 512 * c:512 * (c + 1)],
                                                   ALU.mult, ALU.add)
                    nc.vector.tensor_tensor(y2_bf[J][:, 512 * c:512 * (c + 1)], yd[:],
                                            sz_bf[J][:, 512 * c:512 * (c + 1)], ALU.mult)

        # ============ P7: out_proj partial -> ReduceScatter4 ============
        with ExitStack() as p7:
            wout = p7.enter_context(tc.tile_pool(name="wout", bufs=1))
            w_out_t = [wout.tile([128, D_MODEL], BF16, tag=f"wo{k}", name=f"wo{k}") for k in range(NET)]
            for k in range(NET):
                nc.sync.dma_start(w_out_t[k][:], w_out_T[128 * k:128 * (k + 1), :])
            pps = p7.enter_context(tc.tile_pool(name="p7ps", bufs=4, space="PSUM"))
            sp = p7.enter_context(tc.tile_pool(name="p7s", bufs=4))
            for m in range(8):
                for c in range(NCH):
                    ps = pps.tile([128, 512], F32, tag="ps7", name="ps7")
                    for k in range(NET):
                        nc.tensor.matmul(ps[:], w_out_t[k][:, 128 * m:128 * (m + 1)],
                                         y2_bf[k][:, 512 * c:512 * (c + 1)],
                                         start=(k == 0), stop=(k == NET - 1))
                    ob = sp.tile([128, 512], BF16, tag="ob", name="ob")
                    nc.scalar.copy(ob[:], ps[:])
                    rr = slice(D_MODEL * c + 128 * m, D_MODEL * c + 128 * (m + 1))
                    nc.sync.dma_start(rs_in_h[0][rr, :], ob[:, 0:TQ // 2])
                    nc.sync.dma_start(rs_in_h[1][rr, :], ob[:, TQ // 2:TQ])
            for h in range(2):
                if nocc or nocc_rs:
                    nc.sync.dma_start(rs_out_h[h], rs_in_h[h][0:D_MODEL, :])
                else:
                    nc.gpsimd.collective_compute("ReduceScatter", ALU.add,
                                                 replica_groups=g4,
                                                 ins=[rs_in_h[h]], outs=[rs_out_h[h]])

        mid.close()

        # ============ P8: MLP tail ============
        if ab_no_mlp:
            with tc.tile_pool(name="abz", bufs=1) as abz:
                zt = abz.tile([128, D_MODEL], F32)
                nc.vector.memset(zt[:], 0.0)
                for i in range(TQ // 128):
                    nc.sync.dma_start(out[128 * i:128 * (i + 1), :], zt[:])
        with _skippable(), ExitStack() as p8:
            if ab_no_mlp:
                raise _SkipBlock
            wmlp = p8.enter_context(tc.tile_pool(name="wmlp", bufs=1))
            w_fc_t = [wmlp.tile([128, 2 * D_MODEL], BF16, tag=f"wf{k}", name=f"wf{k}") for k in range(8)]
            for k in range(8):
                nc.sync.dma_start(w_fc_t[k][:], w_fc_T[128 * k:128 * (k + 1), :])
            w_pr_t = [wmlp.tile([128, D_MODEL], BF16, tag=f"wp{k}", name=f"wp{k}") for k in range(16)]
            for k in range(16):
                nc.sync.dma_start(w_pr_t[k][:], w_pr_T[128 * k:128 * (k + 1), :])

            ar = p8.enter_context(tc.tile_pool(name="p8a", bufs=1))
            st = p8.enter_context(tc.tile_pool(name="p8t", bufs=2))
            ppt = p8.enter_context(tc.tile_pool(name="p8pt", bufs=2, space="PSUM"))
            ppm = p8.enter_context(tc.tile_pool(name="p8pm", bufs=2, space="PSUM"))
            pp1 = p8.enter_context(tc.tile_pool(name="p8p1", bufs=1, space="PSUM"))

            TH = TQ // 2
            for th in range(2):
                t0 = TH * th
                x2_T = [ar.tile([128, TH], F32, tag=f"x2T{k}", name=f"x2T{k}")
                        for k in range(8)]
                for k in range(8):
                    nc.sync.dma_start(x2_T[k][:], xqT_dram[128 * k:128 * (k + 1), t0:t0 + TH])
                rsb = [ar.tile([128, TH], BF16, tag=f"rsb{k}", name=f"rsb{k}") for k in range(8)]
                for k in range(8):
                    nc.sync.dma_start(rsb[k][:], rs_out_h[th][128 * k:128 * (k + 1), :])
                    nc.vector.tensor_tensor(x2_T[k][:], x2_T[k][:], rsb[k][:], ALU.add)

                # rmsnorm over features via ones-matmul
                sq = [ar.tile([128, TH], BF16, tag=f"sq{k}", name=f"sq{k}") for k in range(8)]
                for k in range(8):
                    nc.scalar.activation(sq[k][:], x2_T[k][:], AF.Square)
                pss = pp1.tile([1, TH], F32, tag="pss", name="pss")
                for k in range(8):
                    nc.tensor.matmul(pss[:], ones_t[:], sq[k][:], start=(k == 0), stop=(k == 7))
                rrow = st.tile([1, TH], F32, tag="rrow", name="rrow")
                nc.scalar.activation(rrow[:], pss[:], AF.Sqrt, scale=1.0 / D_MODEL,
                                     bias=eps_t[0:1, 0:1])
                rrec = st.tile([1, TH], F32, tag="rrec", name="rrec")
                nc.vector.reciprocal(rrec[:], rrow[:])
                rbf = st.tile([1, TH], BF16, tag="rbf", name="rbf")
                nc.vector.tensor_copy(rbf[:], rrec[:])
                pr2 = pp1.tile([128, TH], F32, tag="pr2", name="pr2")
                nc.tensor.matmul(pr2[:], onesr_t[:], rbf[:], start=True, stop=True)
                x2n = [ar.tile([128, TH], BF16, tag=f"x2n{k}", name=f"x2n{k}") for k in range(8)]
                for k in range(8):
                    nc.vector.tensor_tensor(x2n[k][:], x2_T[k][:], pr2[:], ALU.mult)

                # c_fc + relu^2
                hh_t = [ar.tile([128, TH], BF16, tag=f"hh{k}", name=f"hh{k}") for k in range(16)]
                for m in range(16):
                    pm = ppm.tile([128, TH], F32, tag="pmm", name="pmm")
                    for k in range(8):
                        nc.tensor.matmul(pm[:], w_fc_t[k][:, 128 * m:128 * (m + 1)], x2n[k][:],
                                         start=(k == 0), stop=(k == 7))
                    rl = st.tile([128, TH], BF16, tag="rl", name="rl")
                    nc.scalar.activation(rl[:], pm[:], AF.Relu)
                    nc.vector.tensor_tensor(hh_t[m][:], rl[:], rl[:], ALU.mult)
                # c_proj + residual
                fin = [ar.tile([128, TH], F32, tag=f"fin{k}", name=f"fin{k}") for k in range(8)]
                for m in range(8):
                    pm = ppm.tile([128, TH], F32, tag="pmm", name="pmm")
                    for k in range(16):
                        nc.tensor.matmul(pm[:], w_pr_t[k][:, 128 * m:128 * (m + 1)], hh_t[k][:],
                                         start=(k == 0), stop=(k == 15))
                    nc.vector.tensor_tensor(fin[m][:], x2_T[m][:], pm[:], ALU.add)
                # transpose to token-major + store
                for i in range(TH // 128):
                    for h in range(2):
                        pt = ppt.tile([128, 512], F32, tag="ptx", name="ptx")
                        for q in range(4):
                            m = 4 * h + q
                            nc.tensor.transpose(pt[:, 128 * q:128 * (q + 1)],
                                                fin[m][:, 128 * i:128 * (i + 1)], idf_t[:])
                        ot = st.tile([128, 512], F32, tag="ot", name="ot")
                        nc.scalar.copy(ot[:], pt[:])
                        nc.sync.dma_start(out[t0 + 128 * i:t0 + 128 * (i + 1),
                                              512 * h:512 * (h + 1)], ot[:])

    nc.compile()
    return nc


def _prep_inputs(inputs):
    x = np.asarray(inputs['x'], np.float32)
    in_proj_w = np.asarray(inputs['in_proj_w'], np.float32)
    conv_w = np.asarray(inputs['conv_w'], np.float32)
    conv_b = np.asarray(inputs['conv_b'], np.float32)
    x_proj_w = np.asarray(inputs['x_proj_w'], np.float32)
    dt_proj_w = np.asarray(inputs['dt_proj_w'], np.float32)
    dt_proj_b = np.asarray(inputs['dt_proj_b'], np.float32)
    A_log = np.asarray(inputs['A_log'], np.float32)
    D = np.asarray(inputs['D'], np.float32)
    out_proj_w = np.asarray(inputs['out_proj_w'], np.float32)
    c_fc_w = np.asarray(inputs['c_fc_w'], np.float32)
    c_proj_w = np.asarray(inputs['c_proj_w'], np.float32)

    import ml_dtypes
    bf = lambda a: np.ascontiguousarray(a).astype(ml_dtypes.bfloat16)
    f32 = lambda a: np.ascontiguousarray(a, np.float32)

    r01 = np.zeros((16, 128, 128), np.float32)  # [jm][k, m] = 1 iff k == 8*jm + m%8
    g01 = np.zeros((16, 128, 128), np.float32)  # [jm][k, m] = 1 iff m == 8*jm + k%8
    for jm in range(16):
        for m in range(128):
            r01[jm, 8 * jm + (m % 8), m] = 1.0
            g01[jm, m, 8 * jm + (m % 8)] = 1.0
    s01n = np.zeros((D_STATE, 128), np.float32)
    s01p = np.zeros((D_STATE, 128), np.float32)
    for m in range(128):
        s01n[m // 8, m] = -1.0
        s01p[m // 8, m] = 1.0
    ident = np.eye(128, dtype=np.float32)

    def col_fold(a):
        # (EL,) or (EL, w) -> (128, NET*w): cols [w*k:w*(k+1)] = rows of e-tile k
        a = a.reshape(EL, -1)
        w = a.shape[1]
        o = np.zeros((128, NET * w), np.float32)
        for k in range(NET):
            o[:, w * k:w * (k + 1)] = a[128 * k:128 * (k + 1)]
        return o

    in_maps = []
    for c in range(8):
        b, r = c // 4, c % 4
        sl = slice(EL * r, EL * (r + 1))
        negA_ = np.zeros((128, NJ), np.float32)
        p = np.arange(128)
        for j in range(NJ):
            e = EL * r + 8 * j + (p % 8)
            s = p // 8
            negA_[:, j] = np.exp(A_log[e, s])
        msk0 = np.full((XD, 1), 1.0 if b == 0 else 0.0, np.float32)
        msk1 = np.full((XD, 1), 1.0 if b == 1 else 0.0, np.float32)
        cw = conv_w[sl]
        conv_d = np.zeros((128, NET * D_CONV * 128), np.float32)
        for k in range(NET):
            for tap in range(D_CONV):
                base = 128 * (D_CONV * k + tap)
                conv_d[np.arange(128), base + np.arange(128)] = cw[128 * k:128 * (k + 1), tap]
        in_maps.append({
            'xb': f32(x[b]),
            'xq': f32(x[b][TQ * r:TQ * (r + 1)]),
            'w_in_T': bf(np.concatenate([in_proj_w[sl], in_proj_w[D_INNER:][sl]], 0).T),
            'conv_wc': col_fold(conv_w[sl]),
            'conv_d': bf(conv_d),
            'conv_bc': col_fold(conv_b[sl]),
            'w_xp_T': bf(x_proj_w[:, sl].T),
            'w_dt_T': bf(dt_proj_w[sl].T),
            'dt_bnc': col_fold(-dt_proj_b[sl]),
            'negA': negA_,
            'd_c': col_fold(D[sl]),
            'w_out_T': bf(out_proj_w[:, sl].T),
            'w_fc_T': bf(c_fc_w.T),
            'w_pr_T': bf(c_proj_w.T),
            'r01': bf(r01.reshape(16 * 128, 128)),
            'g01': bf(g01.reshape(16 * 128, 128)),
            's01n': bf(s01n),
            's01p': bf(s01p),
            'ident_bf': bf(ident),
            'ident_f32': f32(ident),
            'ones_bf': bf(np.ones((128, 1), np.float32)),
            'ones_row_bf': bf(np.ones((1, 128), np.float32)),
            'msk0': msk0,
            'msk1': msk1,
        })
    return in_maps


def kernel(**inputs) -> np.ndarray:
    if 'nc' not in _CACHE:
        _CACHE['nc'] = _build()
    nc = _CACHE['nc']
    in_maps = _prep_inputs(inputs)
    res = run_bass_kernel_spmd(nc, in_maps, core_ids=list(range(8)))
    out = np.zeros((B, T, D_MODEL), np.float32)
    for c in range(8):
        b, r = c // 4, c % 4
        out[b, TQ * r:TQ * (r + 1), :] = res.results[c]['out']
    return out



# revision 25
# speedup vs baseline: 1.0634x; 1.0209x over previous
"""Trainium2 Bass kernel for nn_Block_4526895530469 (Mamba block + MLP residual).

Sharding over 8 NeuronCores: core c -> batch b=c//4, channel shard r=c%4
(512 of the 2048 d_inner channels), full T=2048 sequence per core. The
selective scan runs full-T per channel on the Vector engine
(tensor_tensor_scan), so there is no cross-core state chain. Layout for the
scan is (s,e)-pairs on partitions (row p = 8*s + e_local, 16 states x 8
channels per 128-row tile) x time on the free dimension.

Collectives: one zero-padded global AllReduce for the (96, T) x_dbl partial
sums (contraction over the sharded d_inner), and one 4-group ReduceScatter
for the out_proj partials which simultaneously scatters tokens for the
token-parallel MLP tail.
"""
import sys
sys.path.insert(0, '/opt/trn_rl_repo')

import numpy as np
from contextlib import ExitStack

import concourse.bass as bass
from concourse import bacc
import concourse.tile as tile
from concourse import mybir
from concourse.bass_utils import run_bass_kernel_spmd

# The interp (used by Tile's scheduling pass and by test simulation) lacks
# Silu; emulate it: run the existing Sigmoid path, then multiply by the
# scaled/biased input.
from concourse import bass_interp as _bi
from concourse import mybir as _mb

_orig_visit_act = _bi.InstructionExecutor.visit_InstActivation


def _visit_act_with_silu(self, instruction, *a, **kw):
    if instruction.func != _mb.ActivationFunctionType.Silu:
        return _orig_visit_act(self, instruction, *a, **kw)
    import numpy as _np
    assert len(instruction.outs) == 1, "Silu shim: no accum_out support"
    func0 = instruction.func
    try:
        instruction.func = _mb.ActivationFunctionType.Sigmoid
        res = _orig_visit_act(self, instruction, *a, **kw)
    finally:
        instruction.func = func0
    reg_snapshot = kw.get("reg_snapshot")
    inp = self.view_ap(instruction.ins[0], _bi.Direction.READ, instruction,
                       reg_snapshot=reg_snapshot).astype(_np.float32)
    inp = inp.reshape(inp.shape[0], -1)

    def _val(arg):
        if isinstance(arg, _mb.ImmediateValue):
            return arg.value
        v = self.view_ap(arg, _bi.Direction.READ, instruction,
                         reg_snapshot=reg_snapshot).astype(_np.float32)
        return v.reshape(v.shape[0], -1)

    bias = _val(instruction.ins[1])
    scale = _val(instruction.ins[2])
    sx = inp * scale + bias
    out_view = self.view_ap(instruction.outs[0], _bi.Direction.WRITE, instruction,
                            reg_snapshot=reg_snapshot)
    sig = _np.asarray(out_view, dtype=_np.float32).reshape(sx.shape)
    out_view[:] = (sig * sx).reshape(out_view.shape).astype(out_view.dtype)
    return res


_bi.InstructionExecutor.visit_InstActivation = _visit_act_with_silu

F32 = mybir.dt.float32
BF16 = mybir.dt.bfloat16
AF = mybir.ActivationFunctionType
ALU = mybir.AluOpType

D_MODEL, D_INNER, D_STATE, D_CONV, DT_RANK = 1024, 2048, 16, 4, 64
B, T = 2, 2048
EL = D_INNER // 4          # 512 channels per core
NET = EL // 128            # 4 e-tiles
NJ = EL // 8               # 64 scan tiles
NCH = T // 512             # 4 t-chunks
TQ = T // 4                # 512 tokens for the MLP tail
XD = DT_RANK + 2 * D_STATE  # 96
EPS = float(np.finfo(np.float32).eps)

_CACHE = {}


class _SkipBlock(Exception):
    pass


from contextlib import contextmanager


@contextmanager
def _skippable():
    try:
        yield
    except _SkipBlock:
        pass


def _build(nocc=False, gps_mod=2, ar_bf16=True, nocc_ar=False, nocc_rs=False, rs_split=True,
           ab_no_p6=False, ab_scan_tt=False, ab_no_dtur=False, ab_no_p1p2=False,
           ab_no_mlp=False, bb_pool_mod=2):
    nc = bacc.Bacc("TRN2", target_bir_lowering=False, debug=False, num_devices=8)

    def din(name, shape, dt=BF16):
        return nc.dram_tensor(name, list(shape), dt, kind="ExternalInput").ap()

    xb = din("xb", (T, D_MODEL), F32)
    xq = din("xq", (TQ, D_MODEL), F32)
    w_in_T = din("w_in_T", (D_MODEL, 2 * EL))
    conv_wc = din("conv_wc", (128, NET * D_CONV), F32)   # cols [4k:4k+4] = e-tile k
    conv_d = din("conv_d", (128, NET * D_CONV * 128))    # [128,128] diag blocks
    conv_bc = din("conv_bc", (128, NET), F32)
    w_xp_T = din("w_xp_T", (EL, XD))
    w_dt_T = din("w_dt_T", (DT_RANK, EL))
    dt_bnc = din("dt_bnc", (128, NET), F32)              # -dt_proj_b
    negA = din("negA", (128, NJ), F32)                   # exp(A_log) per scan tile col
    d_c = din("d_c", (128, NET), F32)
    w_out_T = din("w_out_T", (EL, D_MODEL))
    w_fc_T = din("w_fc_T", (D_MODEL, 2 * D_MODEL))
    w_pr_T = din("w_pr_T", (2 * D_MODEL, D_MODEL))
    r01 = din("r01", (16 * 128, 128))
    g01 = din("g01", (16 * 128, 128))
    s01n = din("s01n", (D_STATE, 128))
    s01p = din("s01p", (D_STATE, 128))
    ident_bf = din("ident_bf", (128, 128))
    ident_f32 = din("ident_f32", (128, 128), F32)
    ones_bf = din("ones_bf", (128, 1))
    ones_row_bf = din("ones_row_bf", (1, 128))
    msk0 = din("msk0", (XD, 1), F32)
    msk1 = din("msk1", (XD, 1), F32)

    out = nc.dram_tensor("out", [TQ, D_MODEL], F32, kind="ExternalOutput").ap()

    xdbl_in = nc.dram_tensor("xdbl_in", [2 * XD, T], BF16).ap()
    xdbl_out = nc.dram_tensor("xdbl_out", [2 * XD, T], BF16).ap()
    rs_in_h = [nc.dram_tensor(f"rs_in{h}", [4 * D_MODEL, TQ // 2], BF16).ap() for h in range(2)]
    rs_out_h = [nc.dram_tensor(f"rs_out{h}", [D_MODEL, TQ // 2], BF16).ap() for h in range(2)]
    dtu_dram = nc.dram_tensor("dtu_dram", [EL, T], BF16).ap()
    xqT_dram = nc.dram_tensor("xqT_dram", [D_MODEL, TQ], F32).ap()

    g8 = [[0, 1, 2, 3, 4, 5, 6, 7]]
    g4 = [[0, 1, 2, 3], [4, 5, 6, 7]]

    with tile.TileContext(nc) as tc, ExitStack() as top:
        cpool = top.enter_context(tc.tile_pool(name="consts", bufs=1))

        def cload(nm, name_ap, shape, dt=BF16):
            t = cpool.tile(list(shape), dt, tag=nm, name=nm)
            nc.sync.dma_start(t[:], name_ap)
            return t

        negA_t = cload("negA_t", negA, (128, NJ), F32)
        convd_t = cload("convd_t", conv_d, (128, NET * D_CONV * 128))
        convb_t = cload("convb_t", conv_bc, (128, NET), F32)
        dtbn_t = cload("dtbn_t", dt_bnc, (128, NET), F32)
        dcol_t = cload("dcol_t", d_c, (128, NET), F32)
        s01n_t = cload("s01n_t", s01n, (D_STATE, 128))
        s01p_t = cload("s01p_t", s01p, (D_STATE, 128))
        idb_t = cload("idb_t", ident_bf, (128, 128))
        idf_t = cload("idf_t", ident_f32, (128, 128), F32)
        ones_t = cload("ones_t", ones_bf, (128, 1))
        onesr_t = cload("onesr_t", ones_row_bf, (1, 128))
        msk0_t = cload("msk0_t", msk0, (XD, 1), F32)
        msk1_t = cload("msk1_t", msk1, (XD, 1), F32)
        eps_t = cpool.tile([128, 1], F32)
        nc.vector.memset(eps_t[:], EPS)
        r01_t = cpool.tile([128, 16 * 128], BF16)
        for k in range(16):
            nc.sync.dma_start(r01_t[:, 128 * k:128 * (k + 1)], r01[128 * k:128 * (k + 1), :])
        g01_t = cpool.tile([128, 16 * 128], BF16)
        for k in range(16):
            nc.sync.dma_start(g01_t[:, 128 * k:128 * (k + 1)], g01[128 * k:128 * (k + 1), :])

        # long-lived activations through the scan phase (freed before MLP)
        mid = top.enter_context(ExitStack())
        acts = mid.enter_context(tc.tile_pool(name="acts", bufs=1))
        u_bf = [acts.tile([128, T], BF16, tag=f"u{k}", name=f"u{k}") for k in range(NET)]
        sz_bf = [acts.tile([128, T], BF16, tag=f"sz{k}", name=f"sz{k}") for k in range(NET)]
        lnsig_bf = [acts.tile([128, T], BF16, tag=f"lns{k}", name=f"lns{k}") for k in range(NET)]
        brep_t = acts.tile([128, T], BF16, tag="brep", name="brep")
        crep_t = acts.tile([128, T], BF16, tag="crep", name="crep")
        dtlow_bf = acts.tile([DT_RANK, T], BF16, tag="dtlow", name="dtlow")

        # ============ P1+P2: rmsnorm, transpose, in_proj ============
        with ExitStack() as ph:
            winp = ph.enter_context(tc.tile_pool(name="win", bufs=1))
            w_in_t = [winp.tile([128, 2 * EL], BF16, tag=f"wi{k}", name=f"wi{k}") for k in range(8)]
            for k in range(8):
                nc.sync.dma_start(w_in_t[k][:], w_in_T[128 * k:128 * (k + 1), :])

            xinp = ph.enter_context(tc.tile_pool(name="xinz", bufs=1))
            # 3 zero pad columns in front so the causal-conv PE matmuls can
            # read shifted windows without bounds checks
            x_in = [xinp.tile([128, 3 + T], BF16, tag=f"xin{k}", name=f"xin{k}")
                    for k in range(NET)]
            for k in range(NET):
                nc.vector.memset(x_in[k][:, 0:3], 0.0)

            if ab_no_p1p2:
                for k in range(NET):
                    nc.vector.memset(x_in[k][:], 0.01)
                    nc.vector.memset(sz_bf[k][:], 0.01)
            with _skippable(), ExitStack() as p1:
                if ab_no_p1p2:
                    raise _SkipBlock
                xnp = p1.enter_context(tc.tile_pool(name="xnT", bufs=1))
                xn_T = [xnp.tile([128, T], BF16, tag=f"xnT{k}", name=f"xnT{k}") for k in range(8)]
                str_p = p1.enter_context(tc.tile_pool(name="p1s", bufs=3))
                xtmp = p1.enter_context(tc.tile_pool(name="p1x", bufs=6))
                jp = p1.enter_context(tc.tile_pool(name="p1j", bufs=1))
                junk = jp.tile([128, D_MODEL], BF16)
                pps1 = p1.enter_context(tc.tile_pool(name="p1ps", bufs=2, space="PSUM"))

                for c in range(NCH):
                    xnt4 = []
                    for q in range(4):
                        i = 4 * c + q
                        xt = str_p.tile([128, D_MODEL], BF16, tag="xt", name="xt")
                        nc.gpsimd.dma_start(xt[:], xb[128 * i:128 * (i + 1), :])
                        ssq = str_p.tile([128, 1], F32, tag="ssq", name="ssq")
                        nc.scalar.activation(junk[:], xt[:], AF.Square, accum_out=ssq[:])
                        rr = str_p.tile([128, 1], F32, tag="rr", name="rr")
                        nc.scalar.activation(rr[:], ssq[:], AF.Sqrt, scale=1.0 / D_MODEL,
                                             bias=eps_t[:, 0:1])
                        rc = str_p.tile([128, 1], F32, tag="rc", name="rc")
                        nc.vector.reciprocal(rc[:], rr[:])
                        xnt = xtmp.tile([128, D_MODEL], BF16, tag="xnt", name="xnt")
                        nc.vector.tensor_scalar_mul(xnt[:], xt[:], rc[:, 0:1])
                        xnt4.append(xnt)
                    for k in range(8):
                        pt = pps1.tile([128, 512], BF16, tag="pt", name="pt")
                        for q in range(4):
                            nc.tensor.transpose(pt[:, 128 * q:128 * (q + 1)],
                                                xnt4[q][:, 128 * k:128 * (k + 1)],
                                                idb_t[:])
                        nc.vector.tensor_copy(xn_T[k][:, 512 * c:512 * (c + 1)], pt[:])

                # in_proj
                pps2 = p1.enter_context(tc.tile_pool(name="p2ps", bufs=4, space="PSUM"))
                for m in range(8):
                    for c in range(NCH):
                        ps = pps2.tile([128, 512], F32, tag="ps", name="ps")
                        for k in range(8):
                            nc.tensor.matmul(ps[:], w_in_t[k][:, 128 * m:128 * (m + 1)],
                                             xn_T[k][:, 512 * c:512 * (c + 1)],
                                             start=(k == 0), stop=(k == 7))
                        if m < 4:
                            nc.scalar.copy(x_in[m][:, 3 + 512 * c:3 + 512 * (c + 1)], ps[:])
                        else:
                            nc.scalar.activation(sz_bf[m - 4][:, 512 * c:512 * (c + 1)],
                                                 ps[:], AF.Silu)

            # ============ P3: conv (PE diag-matmuls) + silu -> u ============
            with ExitStack() as p3:
                pps3 = p3.enter_context(tc.tile_pool(name="p3ps", bufs=2, space="PSUM"))
                for k in range(NET):
                    for c in range(NCH):
                        ps = pps3.tile([128, 512], F32, tag="psc", name="psc")
                        for tap in range(D_CONV):
                            sh = D_CONV - 1 - tap       # time shift for this tap
                            dcol = 128 * (D_CONV * k + tap)
                            nc.tensor.matmul(ps[:], convd_t[:, dcol:dcol + 128],
                                             x_in[k][:, 3 + 512 * c - sh:3 + 512 * (c + 1) - sh],
                                             start=(tap == 0), stop=(tap == D_CONV - 1))
                        nc.scalar.activation(u_bf[k][:, 512 * c:512 * (c + 1)], ps[:],
                                             AF.Silu, bias=convb_t[:, k:k + 1])

        # ============ P4: x_proj partial -> AllReduce8 -> dtlow/B_rep/C_rep ============
        with ExitStack() as p4:
            wxp = p4.enter_context(tc.tile_pool(name="wxp", bufs=1))
            w_xp_t = [wxp.tile([128, XD], BF16, tag=f"wxp{k}", name=f"wxp{k}") for k in range(NET)]
            for k in range(NET):
                nc.sync.dma_start(w_xp_t[k][:], w_xp_T[128 * k:128 * (k + 1), :])
            pps = p4.enter_context(tc.tile_pool(name="p4ps", bufs=2, space="PSUM"))
            sp = p4.enter_context(tc.tile_pool(name="p4s", bufs=2))
            big = p4.enter_context(tc.tile_pool(name="p4big", bufs=1))
            for c in range(NCH):
                ps = pps.tile([XD, 512], F32, tag="ps4", name="ps4")
                for k in range(NET):
                    nc.tensor.matmul(ps[:], w_xp_t[k][:], u_bf[k][:, 512 * c:512 * (c + 1)],
                                     start=(k == 0), stop=(k == NET - 1))
                t0 = sp.tile([XD, 512], BF16, tag="t0", name="t0")
                nc.vector.tensor_scalar_mul(t0[:], ps[:], msk0_t[:, 0:1])
                t1 = sp.tile([XD, 512], BF16, tag="t1", name="t1")
                nc.vector.tensor_scalar_mul(t1[:], ps[:], msk1_t[:, 0:1])
                nc.sync.dma_start(xdbl_in[0:XD, 512 * c:512 * (c + 1)], t0[:])
                nc.sync.dma_start(xdbl_in[XD:2 * XD, 512 * c:512 * (c + 1)], t1[:])
            if nocc or nocc_ar:
                nc.sync.dma_start(xdbl_out, xdbl_in)
            else:
                nc.gpsimd.collective_compute("AllReduce", ALU.add, replica_groups=g8,
                                             ins=[xdbl_in], outs=[xdbl_out])
            # fill the AllReduce window: transpose the residual token-quarter
            # (f32) now and stage it in DRAM for the MLP tail
            ppq = p4.enter_context(tc.tile_pool(name="p4q", bufs=2, space="PSUM"))
            spq = p4.enter_context(tc.tile_pool(name="p4qs", bufs=2))
            for i in range(TQ // 128):
                xt_ = spq.tile([128, D_MODEL], F32, tag="xq_tm", name="xq_tm")
                nc.sync.dma_start(xt_[:], xq[128 * i:128 * (i + 1), :])
                for h in range(2):
                    ptq = ppq.tile([128, 512], F32, tag="ptq", name="ptq")
                    for q in range(4):
                        k = 4 * h + q
                        nc.tensor.transpose(ptq[:, 128 * q:128 * (q + 1)],
                                            xt_[:, 128 * k:128 * (k + 1)], idf_t[:])
                    otq = spq.tile([128, 512], F32, tag="otq", name="otq")
                    nc.scalar.copy(otq[:], ptq[:])
                    for q in range(4):
                        k = 4 * h + q
                        nc.sync.dma_start(
                            xqT_dram[128 * k:128 * (k + 1), 128 * i:128 * (i + 1)],
                            otq[:, 128 * q:128 * (q + 1)])
            # batch-select each 32-aligned section separately (compute ops
            # cannot start at partition 80)
            def _sel(rows, nrows, out_dt, nm):
                a0 = big.tile([nrows, T], BF16, tag=nm + "a0", name=nm + "a0")
                nc.sync.dma_start(a0[:], xdbl_out[rows:rows + nrows, :])
                a1 = big.tile([nrows, T], BF16, tag=nm + "a1", name=nm + "a1")
                nc.sync.dma_start(a1[:], xdbl_out[XD + rows:XD + rows + nrows, :])
                nc.vector.tensor_scalar_mul(a0[:], a0[:], msk0_t[0:nrows, 0:1])
                o = big.tile([nrows, T], out_dt, tag=nm, name=nm)
                nc.vector.scalar_tensor_tensor(o[:], a1[:], msk1_t[0:nrows, 0:1],
                                               a0[:], ALU.mult, ALU.add)
                return o
            dl = _sel(0, DT_RANK, BF16, "dl")
            nc.vector.tensor_copy(dtlow_bf[:], dl[:])
            b_sb = _sel(DT_RANK, D_STATE, BF16, "b_sb")
            c_sb = _sel(DT_RANK + D_STATE, D_STATE, BF16, "c_sb")
            pps2 = p4.enter_context(tc.tile_pool(name="p4ps2", bufs=2, space="PSUM"))
            for c in range(NCH):
                pb = pps2.tile([128, 512], F32, tag="pb", name="pb")
                nc.tensor.matmul(pb[:], s01n_t[:], b_sb[:, 512 * c:512 * (c + 1)],
                                 start=True, stop=True)
                nc.vector.tensor_copy(brep_t[:, 512 * c:512 * (c + 1)], pb[:])
                pc = pps2.tile([128, 512], F32, tag="pc", name="pc")
                nc.tensor.matmul(pc[:], s01p_t[:], c_sb[:, 512 * c:512 * (c + 1)],
                                 start=True, stop=True)
                nc.vector.tensor_copy(crep_t[:, 512 * c:512 * (c + 1)], pc[:])

        # ============ P5: dt path ============
        with ExitStack() as p5:
            wdt = p5.enter_context(tc.tile_pool(name="wdt", bufs=1))
            w_dt_t = wdt.tile([DT_RANK, EL], BF16)
            nc.sync.dma_start(w_dt_t[:], w_dt_T)
            pps = p5.enter_context(tc.tile_pool(name="p5ps", bufs=4, space="PSUM"))
            sp = p5.enter_context(tc.tile_pool(name="p5s", bufs=2))
            sgp = p5.enter_context(tc.tile_pool(name="p5sg", bufs=1))
            # batch all Sigmoid ops, then all Ln ops: Sigmoid and Ln live in
            # different activation-function tables (1.28us reload each)
            sg_all = [sgp.tile([128, T], F32, tag=f"sg{m}", name=f"sg{m}")
                      for m in range(NET)]
            for m in range(NET):
                for c in range(NCH):
                    ps = pps.tile([128, 512], F32, tag="ps5", name="ps5")
                    nc.tensor.matmul(ps[:], w_dt_t[:, 128 * m:128 * (m + 1)],
                                     dtlow_bf[:, 512 * c:512 * (c + 1)], start=True, stop=True)
                    nc.scalar.activation(sg_all[m][:, 512 * c:512 * (c + 1)], ps[:],
                                         AF.Sigmoid, scale=-1.0, bias=dtbn_t[:, m:m + 1])
            for m in range(NET):
                lns = sp.tile([128, T], F32, tag="lns", name="lns")
                nc.scalar.activation(lns[:], sg_all[m][:], AF.Ln)
                nc.vector.tensor_copy(lnsig_bf[m][:], lns[:])
                dtu = sp.tile([128, T], BF16, tag="dtu", name="dtu")
                nc.vector.tensor_tensor(dtu[:], lns[:], u_bf[m][:], ALU.mult)
                nc.sync.dma_start(dtu_dram[128 * m:128 * (m + 1), :], dtu[:])

        # ============ P6: scan + y-sum + gate ============
        y2p = mid.enter_context(tc.tile_pool(name="y2p", bufs=1))
        y2_bf = [y2p.tile([128, T], BF16, tag=f"y2{k}", name=f"y2{k}") for k in range(NET)]
        if ab_no_p6:
            for k in range(NET):
                nc.vector.memset(y2_bf[k][:], 0.01)
        with _skippable(), ExitStack() as p6:
            if ab_no_p6:
                raise _SkipBlock
            reps = p6.enter_context(tc.tile_pool(name="reps", bufs=2, space="PSUM"))
            yps = p6.enter_context(tc.tile_pool(name="ypsum", bufs=1, space="PSUM"))
            sp = p6.enter_context(tc.tile_pool(name="p6s", bufs=3))
            for J in range(4):
                py = yps.tile([128, T], F32, tag="py", name="py")
                for jj in range(16):
                    j = 16 * J + jj
                    dA = sp.tile([128, T], F32, tag="dA", name="dA")
                    for hf in range(2):
                        pr = reps.tile([128, 1024], F32, tag="pr", name="pr")
                        for q in range(2):
                            c = 2 * hf + q
                            nc.tensor.matmul(pr[:, 512 * q:512 * (q + 1)],
                                             r01_t[:, 128 * jj:128 * (jj + 1)],
                                             lnsig_bf[J][:, 512 * c:512 * (c + 1)],
                                             start=True, stop=True)
                        nc.scalar.activation(dA[:, 1024 * hf:1024 * (hf + 1)], pr[:],
                                             AF.Exp, scale=negA_t[:, j:j + 1])
                    dtur = sp.tile([128, T], BF16, tag="dtur", name="dtur")
                    if ab_no_dtur:
                        nc.vector.tensor_copy(dtur[:], u_bf[J][:])
                    else:
                        src = dtu_dram[128 * J + 8 * jj:128 * J + 8 * jj + 8, :]
                        nc.sync.dma_start(dtur[:], src.unsqueeze(0).broadcast_to([16, 8, T]))
                    # scan is DVE-only (walrus rejects it on Pool); the two
                    # elementwise multiplies can run on GpSimd to unload DVE
                    bb = sp.tile([128, T], BF16, tag="bb", name="bb")
                    # balance the two flexible multiplies: DVE is the scan-bound
                    # engine (2 cyc/el for scans), Pool absorbs ~1.25 of the 2
                    # elementwise passes
                    vb = nc.gpsimd if (bb_pool_mod and j % bb_pool_mod) else nc.vector
                    vb.tensor_tensor(bb[:], dtur[:], brep_t[:], ALU.mult)
                    hh = sp.tile([128, T], BF16, tag="hh", name="hh")
                    if ab_scan_tt:
                        nc.vector.tensor_tensor(hh[:], dA[:], bb[:], ALU.mult)
                    else:
                        nc.vector.tensor_tensor_scan(hh[:], dA[:], bb[:], 0.0, ALU.mult, ALU.add)
                    # C-multiply is off the scan-to-scan chain: mostly GpSimd,
                    # 1/3 back on DVE so Pool doesn't become the phase bound
                    ve = (nc.gpsimd if (j % 4) else nc.vector) if gps_mod else nc.vector
                    t1 = sp.tile([128, T], BF16, tag="t1", name="t1")
                    ve.tensor_tensor(t1[:], hh[:], crep_t[:], ALU.mult)
                    for c in range(NCH):
                        nc.tensor.matmul(py[:, 512 * c:512 * (c + 1)],
                                         g01_t[:, 128 * jj:128 * (jj + 1)],
                                         t1[:, 512 * c:512 * (c + 1)],
                                         start=(jj == 0), stop=(jj == 15))
                for c in range(NCH):
                    yd = sp.tile([128, 512], F32, tag="yd", name="yd")
                    nc.vector.scalar_tensor_tensor(yd[:], u_bf[J][:, 512 * c:512 * (c + 1)],
                                                   dcol_t[:, J:J + 1],
                                                   py[:, 512 * c:512 * (c + 1)],
                                                   ALU.mult, ALU.add)
                    nc.vector.tensor_tensor(y2_bf[J][:, 512 * c:512 * (c + 1)], yd[:],
                                            sz_bf[J][:, 512 * c:512 * (c + 1)], ALU.mult)

        # ============ P7: out_proj partial -> ReduceScatter4 ============
        with ExitStack() as p7:
            wout = p7.enter_context(tc.tile_pool(name="wout", bufs=1))
            w_out_t = [wout.tile([128, D_MODEL], BF16, tag=f"wo{k}", name=f"wo{k}") for k in range(NET)]
            for k in range(NET):
                nc.sync.dma_start(w_out_t[k][:], w_out_T[128 * k:128 * (k + 1), :])
            pps = p7.enter_context(tc.tile_pool(name="p7ps", bufs=4, space="PSUM"))
            sp = p7.enter_context(tc.tile_pool(name="p7s", bufs=4))
            for m in range(8):
                for c in range(NCH):
                    ps = pps.tile([128, 512], F32, tag="ps7", name="ps7")
                    for k in range(NET):
                        nc.tensor.matmul(ps[:], w_out_t[k][:, 128 * m:128 * (m + 1)],
                                         y2_bf[k][:, 512 * c:512 * (c + 1)],
                                         start=(k == 0), stop=(k == NET - 1))
                    ob = sp.tile([128, 512], BF16, tag="ob", name="ob")
                    nc.scalar.copy(ob[:], ps[:])
                    rr = slice(D_MODEL * c + 128 * m, D_MODEL * c + 128 * (m + 1))
                    nc.sync.dma_start(rs_in_h[0][rr, :], ob[:, 0:TQ // 2])
                    nc.sync.dma_start(rs_in_h[1][rr, :], ob[:, TQ // 2:TQ])
            for h in range(2):
                if nocc or nocc_rs:
                    nc.sync.dma_start(rs_out_h[h], rs_in_h[h][0:D_MODEL, :])
                else:
                    nc.gpsimd.collective_compute("ReduceScatter", ALU.add,
                                                 replica_groups=g4,
                                                 ins=[rs_in_h[h]], outs=[rs_out_h[h]])

        mid.close()

        # ============ P8: MLP tail ============
        if ab_no_mlp:
            with tc.tile_pool(name="abz", bufs=1) as abz:
                zt = abz.tile([128, D_MODEL], F32)
                nc.vector.memset(zt[:], 0.0)
                for i in range(TQ // 128):
                    nc.sync.dma_start(out[128 * i:128 * (i + 1), :], zt[:])
        with _skippable(), ExitStack() as p8:
            if ab_no_mlp:
                raise _SkipBlock
            wmlp = p8.enter_context(tc.tile_pool(name="wmlp", bufs=1))
            w_fc_t = [wmlp.tile([128, 2 * D_MODEL], BF16, tag=f"wf{k}", name=f"wf{k}") for k in range(8)]
            for k in range(8):
                nc.sync.dma_start(w_fc_t[k][:], w_fc_T[128 * k:128 * (k + 1), :])
            w_pr_t = [wmlp.tile([128, D_MODEL], BF16, tag=f"wp{k}", name=f"wp{k}") for k in range(16)]
            for k in range(16):
                nc.sync.dma_start(w_pr_t[k][:], w_pr_T[128 * k:128 * (k + 1), :])

            ar = p8.enter_context(tc.tile_pool(name="p8a", bufs=1))
            st = p8.enter_context(tc.tile_pool(name="p8t", bufs=2))
            ppt = p8.enter_context(tc.tile_pool(name="p8pt", bufs=2, space="PSUM"))
            ppm = p8.enter_context(tc.tile_pool(name="p8pm", bufs=2, space="PSUM"))
            pp1 = p8.enter_context(tc.tile_pool(name="p8p1", bufs=1, space="PSUM"))

            TH = TQ // 2
            for th in range(2):
                t0 = TH * th
                x2_T = [ar.tile([128, TH], F32, tag=f"x2T{k}", name=f"x2T{k}")
                        for k in range(8)]
                for k in range(8):
                    nc.sync.dma_start(x2_T[k][:], xqT_dram[128 * k:128 * (k + 1), t0:t0 + TH])
                rsb = [ar.tile([128, TH], BF16, tag=f"rsb{k}", name=f"rsb{k}") for k in range(8)]
                for k in range(8):
                    nc.sync.dma_start(rsb[k][:], rs_out_h[th][128 * k:128 * (k + 1), :])
                    nc.vector.tensor_tensor(x2_T[k][:], x2_T[k][:], rsb[k][:], ALU.add)

                # rmsnorm over features via ones-matmul
                sq = [ar.tile([128, TH], BF16, tag=f"sq{k}", name=f"sq{k}") for k in range(8)]
                for k in range(8):
                    nc.scalar.activation(sq[k][:], x2_T[k][:], AF.Square)
                pss = pp1.tile([1, TH], F32, tag="pss", name="pss")
                for k in range(8):
                    nc.tensor.matmul(pss[:], ones_t[:], sq[k][:], start=(k == 0), stop=(k == 7))
                rrow = st.tile([1, TH], F32, tag="rrow", name="rrow")
                nc.scalar.activation(rrow[:], pss[:], AF.Sqrt, scale=1.0 / D_MODEL,
                                     bias=eps_t[0:1, 0:1])
                rrec = st.tile([1, TH], F32, tag="rrec", name="rrec")
                nc.vector.reciprocal(rrec[:], rrow[:])
                rbf = st.tile([1, TH], BF16, tag="rbf", name="rbf")
                nc.vector.tensor_copy(rbf[:], rrec[:])
                pr2 = pp1.tile([128, TH], F32, tag="pr2", name="pr2")
                nc.tensor.matmul(pr2[:], onesr_t[:], rbf[:], start=True, stop=True)
                x2n = [ar.tile([128, TH], BF16, tag=f"x2n{k}", name=f"x2n{k}") for k in range(8)]
                for k in range(8):
                    nc.vector.tensor_tensor(x2n[k][:], x2_T[k][:], pr2[:], ALU.mult)

                # c_fc + relu^2
                hh_t = [ar.tile([128, TH], BF16, tag=f"hh{k}", name=f"hh{k}") for k in range(16)]
                for m in range(16):
                    pm = ppm.tile([128, TH], F32, tag="pmm", name="pmm")
                    for k in range(8):
                        nc.tensor.matmul(pm[:], w_fc_t[k][:, 128 * m:128 * (m + 1)], x2n[k][:],
                                         start=(k == 0), stop=(k == 7))
                    rl = st.tile([128, TH], BF16, tag="rl", name="rl")
                    nc.scalar.activation(rl[:], pm[:], AF.Relu)
                    nc.vector.tensor_tensor(hh_t[m][:], rl[:], rl[:], ALU.mult)
                # c_proj + residual
                fin = [ar.tile([128, TH], F32, tag=f"fin{k}", name=f"fin{k}") for k in range(8)]
                for m in range(8):
                    pm = ppm.tile([128, TH], F32, tag="pmm", name="pmm")
                    for k in range(16):
                        nc.tensor.matmul(pm[:], w_pr_t[k][:, 128 * m:128 * (m + 1)], hh_t[k][:],
                                         start=(k == 0), stop=(k == 15))
                    nc.vector.tensor_tensor(fin[m][:], x2_T[m][:], pm[:], ALU.add)
                # transpose to token-major + store
                for i in range(TH // 128):
                    for h in range(2):
                        pt = ppt.tile([128, 512], F32, tag="ptx", name="ptx")
                        for q in range(4):
                            m = 4 * h + q
                            nc.tensor.transpose(pt[:, 128 * q:128 * (q + 1)],
                                                fin[m][:, 128 * i:128 * (i + 1)], idf_t[:])
                        ot = st.tile([128, 512], F32, tag="ot", name="ot")
                        nc.scalar.copy(ot[:], pt[:])
                        nc.sync.dma_start(out[t0 + 128 * i:t0 + 128 * (i + 1),
                                              512 * h:512 * (h + 1)], ot[:])

    nc.compile()
    return nc


def _prep_inputs(inputs):
    x = np.asarray(inputs['x'], np.float32)
    in_proj_w = np.asarray(inputs['in_proj_w'], np.float32)
    conv_w = np.asarray(inputs['conv_w'], np.float32)
    conv_b = np.asarray(inputs['conv_b'], np.float32)
    x_proj_w = np.asarray(inputs['x_proj_w'], np.float32)
    dt_proj_w = np.asarray(inputs['dt_proj_w'], np.float32)
    dt_proj_b = np.asarray(inputs['dt_proj_b'], np.float32)
    A_log = np.asarray(inputs['A_log'], np.float32)
    D = np.asarray(inputs['D'], np.float32)
    out_proj_w = np.asarray(inputs['out_proj_w'], np.float32)
    c_fc_w = np.asarray(inputs['c_fc_w'], np.float32)
    c_proj_w = np.asarray(inputs['c_proj_w'], np.float32)

    import ml_dtypes
    bf = lambda a: np.ascontiguousarray(a).astype(ml_dtypes.bfloat16)
    f32 = lambda a: np.ascontiguousarray(a, np.float32)

    r01 = np.zeros((16, 128, 128), np.float32)  # [jm][k, m] = 1 iff k == 8*jm + m%8
    g01 = np.zeros((16, 128, 128), np.float32)  # [jm][k, m] = 1 iff m == 8*jm + k%8
    for jm in range(16):
        for m in range(128):
            r01[jm, 8 * jm + (m % 8), m] = 1.0
            g01[jm, m, 8 * jm + (m % 8)] = 1.0
    s01n = np.zeros((D_STATE, 128), np.float32)
    s01p = np.zeros((D_STATE, 128), np.float32)
    for m in range(128):
        s01n[m // 8, m] = -1.0
        s01p[m // 8, m] = 1.0
    ident = np.eye(128, dtype=np.float32)

    def col_fold(a):
        # (EL,) or (EL, w) -> (128, NET*w): cols [w*k:w*(k+1)] = rows of e-tile k
        a = a.reshape(EL, -1)
        w = a.shape[1]
        o = np.zeros((128, NET * w), np.float32)
        for k in range(NET):
            o[:, w * k:w * (k + 1)] = a[128 * k:128 * (k + 1)]
        return o

    in_maps = []
    for c in range(8):
        b, r = c // 4, c % 4
        sl = slice(EL * r, EL * (r + 1))
        negA_ = np.zeros((128, NJ), np.float32)
        p = np.arange(128)
        for j in range(NJ):
            e = EL * r + 8 * j + (p % 8)
            s = p // 8
            negA_[:, j] = np.exp(A_log[e, s])
        msk0 = np.full((XD, 1), 1.0 if b == 0 else 0.0, np.float32)
        msk1 = np.full((XD, 1), 1.0 if b == 1 else 0.0, np.float32)
        cw = conv_w[sl]
        conv_d = np.zeros((128, NET * D_CONV * 128), np.float32)
        for k in range(NET):
            for tap in range(D_CONV):
                base = 128 * (D_CONV * k + tap)
                conv_d[np.arange(128), base + np.arange(128)] = cw[128 * k:128 * (k + 1), tap]
        in_maps.append({
            'xb': f32(x[b]),
            'xq': f32(x[b][TQ * r:TQ * (r + 1)]),
            'w_in_T': bf(np.concatenate([in_proj_w[sl], in_proj_w[D_INNER:][sl]], 0).T),
            'conv_wc': col_fold(conv_w[sl]),
            'conv_d': bf(conv_d),
            'conv_bc': col_fold(conv_b[sl]),
            'w_xp_T': bf(x_proj_w[:, sl].T),
            'w_dt_T': bf(dt_proj_w[sl].T),
            'dt_bnc': col_fold(-dt_proj_b[sl]),
            'negA': negA_,
            'd_c': col_fold(D[sl]),
            'w_out_T': bf(out_proj_w[:, sl].T),
            'w_fc_T': bf(c_fc_w.T),
            'w_pr_T': bf(c_proj_w.T),
            'r01': bf(r01.reshape(16 * 128, 128)),
            'g01': bf(g01.reshape(16 * 128, 128)),
            's01n': bf(s01n),
            's01p': bf(s01p),
            'ident_bf': bf(ident),
            'ident_f32': f32(ident),
            'ones_bf': bf(np.ones((128, 1), np.float32)),
            'ones_row_bf': bf(np.ones((1, 128), np.float32)),
            'msk0': msk0,
            'msk1': msk1,
        })
    return in_maps


def kernel(**inputs) -> np.ndarray:
    if 'nc' not in _CACHE:
        _CACHE['nc'] = _build()
    nc = _CACHE['nc']
    in_maps = _prep_inputs(inputs)
    res = run_bass_kernel_spmd(nc, in_maps, core_ids=list(range(8)))
    out = np.zeros((B, T, D_MODEL), np.float32)
    for c in range(8):
        b, r = c // 4, c % 4
        out[b, TQ * r:TQ * (r + 1), :] = res.results[c]['out']
    return out



# revision 43
# speedup vs baseline: 1.1140x; 1.0476x over previous
"""Trainium2 Bass kernel for nn_Block_4526895530469 (Mamba block + MLP residual).

Sharding over 8 NeuronCores: core c -> batch b=c//4, channel shard r=c%4
(512 of the 2048 d_inner channels), full T=2048 sequence per core. The
selective scan runs full-T per channel on the Vector engine
(tensor_tensor_scan), so there is no cross-core state chain. Layout for the
scan is (s,e)-pairs on partitions (row p = 8*s + e_local, 16 states x 8
channels per 128-row tile) x time on the free dimension.

Collectives: one zero-padded global AllReduce for the (96, T) x_dbl partial
sums (contraction over the sharded d_inner), and one 4-group ReduceScatter
for the out_proj partials which simultaneously scatters tokens for the
token-parallel MLP tail.
"""
import sys
sys.path.insert(0, '/opt/trn_rl_repo')

import numpy as np
from contextlib import ExitStack

import concourse.bass as bass
from concourse import bacc
import concourse.tile as tile
from concourse import mybir
from concourse.bass_utils import run_bass_kernel_spmd

# The interp (used by Tile's scheduling pass and by test simulation) lacks
# Silu; emulate it: run the existing Sigmoid path, then multiply by the
# scaled/biased input.
from concourse import bass_interp as _bi
from concourse import mybir as _mb

_orig_visit_act = _bi.InstructionExecutor.visit_InstActivation


def _visit_act_with_silu(self, instruction, *a, **kw):
    if instruction.func != _mb.ActivationFunctionType.Silu:
        return _orig_visit_act(self, instruction, *a, **kw)
    import numpy as _np
    assert len(instruction.outs) == 1, "Silu shim: no accum_out support"
    func0 = instruction.func
    try:
        instruction.func = _mb.ActivationFunctionType.Sigmoid
        res = _orig_visit_act(self, instruction, *a, **kw)
    finally:
        instruction.func = func0
    reg_snapshot = kw.get("reg_snapshot")
    inp = self.view_ap(instruction.ins[0], _bi.Direction.READ, instruction,
                       reg_snapshot=reg_snapshot).astype(_np.float32)
    inp = inp.reshape(inp.shape[0], -1)

    def _val(arg):
        if isinstance(arg, _mb.ImmediateValue):
            return arg.value
        v = self.view_ap(arg, _bi.Direction.READ, instruction,
                         reg_snapshot=reg_snapshot).astype(_np.float32)
        return v.reshape(v.shape[0], -1)

    bias = _val(instruction.ins[1])
    scale = _val(instruction.ins[2])
    sx = inp * scale + bias
    out_view = self.view_ap(instruction.outs[0], _bi.Direction.WRITE, instruction,
                            reg_snapshot=reg_snapshot)
    sig = _np.asarray(out_view, dtype=_np.float32).reshape(sx.shape)
    out_view[:] = (sig * sx).reshape(out_view.shape).astype(out_view.dtype)
    return res


_bi.InstructionExecutor.visit_InstActivation = _visit_act_with_silu

F32 = mybir.dt.float32
BF16 = mybir.dt.bfloat16
FP8 = mybir.dt.float8e4
AF = mybir.ActivationFunctionType
ALU = mybir.AluOpType

D_MODEL, D_INNER, D_STATE, D_CONV, DT_RANK = 1024, 2048, 16, 4, 64
B, T = 2, 2048
EL = D_INNER // 4          # 512 channels per core
NET = EL // 128            # 4 e-tiles
NJ = EL // 8               # 64 scan tiles
NCH = T // 512             # 4 t-chunks
TQ = T // 4                # 512 tokens for the MLP tail
XD = DT_RANK + 2 * D_STATE  # 96
EPS = float(np.finfo(np.float32).eps)

_CACHE = {}


class _SkipBlock(Exception):
    pass


from contextlib import contextmanager


@contextmanager
def _skippable():
    try:
        yield
    except _SkipBlock:
        pass


def _build(nocc=False, gps_mod=2, ar_bf16=True, nocc_ar=False, nocc_rs=False, rs_split=True,
           ab_no_p6=False, ab_scan_tt=False, ab_no_dtur=False, ab_no_p1p2=False,
           ab_no_mlp=False, bb_pool_mod=2):
    nc = bacc.Bacc("TRN2", target_bir_lowering=False, debug=False, num_devices=8)

    def din(name, shape, dt=BF16):
        return nc.dram_tensor(name, list(shape), dt, kind="ExternalInput").ap()

    xb = din("xb", (T, D_MODEL), F32)
    xq = din("xq", (TQ, D_MODEL), F32)
    w_in_f8 = din("w_in_f8", (D_MODEL, 2 * EL), FP8)   # pre-scaled x256
    conv_wc = din("conv_wc", (128, NET * D_CONV), F32)   # cols [4k:4k+4] = e-tile k
    conv_d = din("conv_d", (128, NET * D_CONV * 128))    # [128,128] diag blocks
    conv_bc = din("conv_bc", (128, NET), F32)
    w_xp_T = din("w_xp_T", (EL, XD))
    w_dt_T = din("w_dt_T", (DT_RANK, EL))
    dt_bnc = din("dt_bnc", (128, NET), F32)              # -dt_proj_b
    negA = din("negA", (128, NJ), F32)                   # exp(A_log) per scan tile col
    d_c = din("d_c", (128, NET), F32)
    w_out_T = din("w_out_T", (EL, D_MODEL))
    w_fc_T = din("w_fc_T", (D_MODEL, 2 * D_MODEL))
    w_pr_T = din("w_pr_T", (2 * D_MODEL, D_MODEL))
    r01 = din("r01", (16 * 128, 128))
    g01 = din("g01", (16 * 128, 128))
    s01n = din("s01n", (D_STATE, 128))
    s01p = din("s01p", (D_STATE, 128))
    ident_bf = din("ident_bf", (128, 128))
    ident_f32 = din("ident_f32", (128, 128), F32)
    ones_bf = din("ones_bf", (128, 1))
    ones_row_bf = din("ones_row_bf", (1, 128))
    msk0 = din("msk0", (XD, 1), F32)
    msk1 = din("msk1", (XD, 1), F32)

    out = nc.dram_tensor("out", [TQ, D_MODEL], F32, kind="ExternalOutput").ap()

    xdbl_in = nc.dram_tensor("xdbl_in", [2 * XD, T], BF16).ap()
    xdbl_out = nc.dram_tensor("xdbl_out", [2 * XD, T], BF16).ap()
    rs_in_h = [nc.dram_tensor(f"rs_in{h}", [4 * D_MODEL, TQ // 2], BF16).ap() for h in range(2)]
    rs_out_h = [nc.dram_tensor(f"rs_out{h}", [D_MODEL, TQ // 2], BF16).ap() for h in range(2)]
    dtu_dram = nc.dram_tensor("dtu_dram", [EL, T], BF16).ap()
    xqT_dram = nc.dram_tensor("xqT_dram", [D_MODEL, TQ], F32).ap()

    g8 = [[0, 1, 2, 3, 4, 5, 6, 7]]
    g4 = [[0, 1, 2, 3], [4, 5, 6, 7]]

    with tile.TileContext(nc) as tc, ExitStack() as top:
        cpool = top.enter_context(tc.tile_pool(name="consts", bufs=1))

        def cload(nm, name_ap, shape, dt=BF16):
            t = cpool.tile(list(shape), dt, tag=nm, name=nm)
            nc.sync.dma_start(t[:], name_ap)
            return t

        negA_t = cload("negA_t", negA, (128, NJ), F32)
        convd_t = cload("convd_t", conv_d, (128, NET * D_CONV * 128))
        convb_t = cload("convb_t", conv_bc, (128, NET), F32)
        dtbn_t = cload("dtbn_t", dt_bnc, (128, NET), F32)
        dcol_t = cload("dcol_t", d_c, (128, NET), F32)
        s01n_t = cload("s01n_t", s01n, (D_STATE, 128))
        s01p_t = cload("s01p_t", s01p, (D_STATE, 128))
        idb_t = cload("idb_t", ident_bf, (128, 128))
        idf_t = cload("idf_t", ident_f32, (128, 128), F32)
        ones_t = cload("ones_t", ones_bf, (128, 1))
        onesr_t = cload("onesr_t", ones_row_bf, (1, 128))
        msk0_t = cload("msk0_t", msk0, (XD, 1), F32)
        msk1_t = cload("msk1_t", msk1, (XD, 1), F32)
        eps_t = cpool.tile([128, 1], F32)
        nc.vector.memset(eps_t[:], EPS)
        eps256_t = cpool.tile([128, 1], F32)
        nc.vector.memset(eps256_t[:], EPS / 256.0)
        r01_t = cpool.tile([128, 16 * 128], BF16)
        for k in range(16):
            nc.sync.dma_start(r01_t[:, 128 * k:128 * (k + 1)], r01[128 * k:128 * (k + 1), :])
        g01_t = cpool.tile([128, 16 * 128], BF16)
        for k in range(16):
            nc.sync.dma_start(g01_t[:, 128 * k:128 * (k + 1)], g01[128 * k:128 * (k + 1), :])

        # long-lived activations through the scan phase (freed before MLP)
        mid = top.enter_context(ExitStack())
        acts = mid.enter_context(tc.tile_pool(name="acts", bufs=1))
        u_bf = [acts.tile([128, T], BF16, tag=f"u{k}", name=f"u{k}") for k in range(NET)]
        sz_bf = [acts.tile([128, T], BF16, tag=f"sz{k}", name=f"sz{k}") for k in range(NET)]
        lnsig_bf = [acts.tile([128, T], BF16, tag=f"lns{k}", name=f"lns{k}") for k in range(NET)]
        brep_t = acts.tile([128, T], BF16, tag="brep", name="brep")
        crep_t = acts.tile([128, T], BF16, tag="crep", name="crep")
        dtlow_bf = acts.tile([DT_RANK, T], BF16, tag="dtlow", name="dtlow")

        # ============ P1+P2: rmsnorm, transpose, in_proj ============
        with ExitStack() as ph:
            winp = ph.enter_context(tc.tile_pool(name="win", bufs=1))
            # fp8 DoubleRow layout: k-tiles on a free axis so one AP can
            # address a (2k, 2k+1) pair for the double-pumped matmul
            w_in_t = winp.tile([128, 8, 2 * EL], FP8, tag="wi8", name="wi8")
            for k in range(8):
                nc.sync.dma_start(w_in_t[:, k, :], w_in_f8[128 * k:128 * (k + 1), :])

            xinp = ph.enter_context(tc.tile_pool(name="xinz", bufs=1))
            # 3 zero pad columns in front so the causal-conv PE matmuls can
            # read shifted windows without bounds checks
            x_in = [xinp.tile([128, 3 + T], BF16, tag=f"xin{k}", name=f"xin{k}")
                    for k in range(NET)]
            for k in range(NET):
                nc.vector.memset(x_in[k][:, 0:3], 0.0)

            if ab_no_p1p2:
                for k in range(NET):
                    nc.vector.memset(x_in[k][:], 0.01)
                    nc.vector.memset(sz_bf[k][:], 0.01)
            with _skippable(), ExitStack() as p1:
                if ab_no_p1p2:
                    raise _SkipBlock
                xnp = p1.enter_context(tc.tile_pool(name="xnT", bufs=1))
                xn8 = xnp.tile([128, 8, T], FP8, tag="xn8", name="xn8")
                str_p = p1.enter_context(tc.tile_pool(name="p1s", bufs=3))
                xtmp = p1.enter_context(tc.tile_pool(name="p1x", bufs=6))
                jp = p1.enter_context(tc.tile_pool(name="p1j", bufs=1))
                junk = jp.tile([128, D_MODEL], BF16)
                pps1 = p1.enter_context(tc.tile_pool(name="p1ps", bufs=2, space="PSUM"))

                for c in range(NCH):
                    xnt4 = []
                    for q in range(4):
                        i = 4 * c + q
                        xt = str_p.tile([128, D_MODEL], BF16, tag="xt", name="xt")
                        nc.gpsimd.dma_start(xt[:], xb[128 * i:128 * (i + 1), :])
                        ssq = str_p.tile([128, 1], F32, tag="ssq", name="ssq")
                        nc.scalar.activation(junk[:], xt[:], AF.Square, accum_out=ssq[:])
                        rr = str_p.tile([128, 1], F32, tag="rr", name="rr")
                        nc.scalar.activation(rr[:], ssq[:], AF.Sqrt, scale=1.0 / D_MODEL,
                                             bias=eps_t[:, 0:1])
                        rc = str_p.tile([128, 1], F32, tag="rc", name="rc")
                        nc.vector.reciprocal(rc[:], rr[:])
                        xnt = xtmp.tile([128, D_MODEL], BF16, tag="xnt", name="xnt")
                        nc.vector.tensor_scalar_mul(xnt[:], xt[:], rc[:, 0:1])
                        xnt4.append(xnt)
                    for k in range(8):
                        pt = pps1.tile([128, 512], BF16, tag="pt", name="pt")
                        for q in range(4):
                            nc.tensor.transpose(pt[:, 128 * q:128 * (q + 1)],
                                                xnt4[q][:, 128 * k:128 * (k + 1)],
                                                idb_t[:])
                        # cast to fp8 with a x16 range shift (fp8e4 min-normal
                        # is 2^-6; unscaled unit-RMS values lose mantissa)
                        nc.scalar.activation(xn8[:, k, 512 * c:512 * (c + 1)], pt[:],
                                             AF.Copy, scale=16.0)

                # in_proj: fp8 DoubleRow (two 128-deep k-tiles per matmul)
                pps2 = p1.enter_context(tc.tile_pool(name="p2ps", bufs=4, space="PSUM"))
                UNSC = 1.0 / (256.0 * 16.0)
                for m in range(8):
                    for c in range(NCH):
                        ps = pps2.tile([128, 512], F32, tag="ps", name="ps")
                        for p2 in range(4):
                            nc.tensor.matmul(ps[:], w_in_t[:, 2 * p2:2 * p2 + 2,
                                                          128 * m:128 * (m + 1)],
                                             xn8[:, 2 * p2:2 * p2 + 2,
                                                 512 * c:512 * (c + 1)],
                                             start=(p2 == 0), stop=(p2 == 3),
                                             perf_mode=mybir.MatmulPerfMode.DoubleRow)
                        if m < 4:
                            nc.scalar.activation(x_in[m][:, 3 + 512 * c:3 + 512 * (c + 1)],
                                                 ps[:], AF.Copy, scale=UNSC)
                        else:
                            nc.scalar.activation(sz_bf[m - 4][:, 512 * c:512 * (c + 1)],
                                                 ps[:], AF.Silu, scale=UNSC)

            # ============ P3: conv (PE diag-matmuls) + silu -> u ============
            with ExitStack() as p3:
                pps3 = p3.enter_context(tc.tile_pool(name="p3ps", bufs=2, space="PSUM"))
                for k in range(NET):
                    for c in range(NCH):
                        ps = pps3.tile([128, 512], F32, tag="psc", name="psc")
                        for tap in range(D_CONV):
                            sh = D_CONV - 1 - tap       # time shift for this tap
                            dcol = 128 * (D_CONV * k + tap)
                            nc.tensor.matmul(ps[:], convd_t[:, dcol:dcol + 128],
                                             x_in[k][:, 3 + 512 * c - sh:3 + 512 * (c + 1) - sh],
                                             start=(tap == 0), stop=(tap == D_CONV - 1))
                        nc.scalar.activation(u_bf[k][:, 512 * c:512 * (c + 1)], ps[:],
                                             AF.Silu, bias=convb_t[:, k:k + 1])

        # ============ P4: x_proj partial -> AllReduce8 -> dtlow/B_rep/C_rep ============
        with ExitStack() as p4:
            wxp = p4.enter_context(tc.tile_pool(name="wxp", bufs=1))
            w_xp_t = [wxp.tile([128, XD], BF16, tag=f"wxp{k}", name=f"wxp{k}") for k in range(NET)]
            for k in range(NET):
                nc.sync.dma_start(w_xp_t[k][:], w_xp_T[128 * k:128 * (k + 1), :])
            pps = p4.enter_context(tc.tile_pool(name="p4ps", bufs=2, space="PSUM"))
            sp = p4.enter_context(tc.tile_pool(name="p4s", bufs=2))
            big = p4.enter_context(tc.tile_pool(name="p4big", bufs=1))
            for c in range(NCH):
                ps = pps.tile([XD, 512], F32, tag="ps4", name="ps4")
                for k in range(NET):
                    nc.tensor.matmul(ps[:], w_xp_t[k][:], u_bf[k][:, 512 * c:512 * (c + 1)],
                                     start=(k == 0), stop=(k == NET - 1))
                t0 = sp.tile([XD, 512], BF16, tag="t0", name="t0")
                nc.vector.tensor_scalar_mul(t0[:], ps[:], msk0_t[:, 0:1])
                t1 = sp.tile([XD, 512], BF16, tag="t1", name="t1")
                nc.vector.tensor_scalar_mul(t1[:], ps[:], msk1_t[:, 0:1])
                nc.sync.dma_start(xdbl_in[0:XD, 512 * c:512 * (c + 1)], t0[:])
                nc.sync.dma_start(xdbl_in[XD:2 * XD, 512 * c:512 * (c + 1)], t1[:])
            if nocc or nocc_ar:
                nc.sync.dma_start(xdbl_out, xdbl_in)
            else:
                nc.gpsimd.collective_compute("AllReduce", ALU.add, replica_groups=g8,
                                             ins=[xdbl_in], outs=[xdbl_out])
            # fill the AllReduce window: transpose the residual token-quarter
            # (f32) now and stage it in DRAM for the MLP tail
            ppq = p4.enter_context(tc.tile_pool(name="p4q", bufs=2, space="PSUM"))
            spq = p4.enter_context(tc.tile_pool(name="p4qs", bufs=2))
            for i in range(TQ // 128):
                xt_ = spq.tile([128, D_MODEL], F32, tag="xq_tm", name="xq_tm")
                nc.sync.dma_start(xt_[:], xq[128 * i:128 * (i + 1), :])
                for h in range(2):
                    ptq = ppq.tile([128, 512], F32, tag="ptq", name="ptq")
                    for q in range(4):
                        k = 4 * h + q
                        nc.tensor.transpose(ptq[:, 128 * q:128 * (q + 1)],
                                            xt_[:, 128 * k:128 * (k + 1)], idf_t[:])
                    otq = spq.tile([128, 512], F32, tag="otq", name="otq")
                    nc.scalar.copy(otq[:], ptq[:])
                    for q in range(4):
                        k = 4 * h + q
                        nc.sync.dma_start(
                            xqT_dram[128 * k:128 * (k + 1), 128 * i:128 * (i + 1)],
                            otq[:, 128 * q:128 * (q + 1)])
            # batch-select each 32-aligned section separately (compute ops
            # cannot start at partition 80)
            def _sel(rows, nrows, out_dt, nm):
                a0 = big.tile([nrows, T], BF16, tag=nm + "a0", name=nm + "a0")
                nc.sync.dma_start(a0[:], xdbl_out[rows:rows + nrows, :])
                a1 = big.tile([nrows, T], BF16, tag=nm + "a1", name=nm + "a1")
                nc.sync.dma_start(a1[:], xdbl_out[XD + rows:XD + rows + nrows, :])
                nc.vector.tensor_scalar_mul(a0[:], a0[:], msk0_t[0:nrows, 0:1])
                o = big.tile([nrows, T], out_dt, tag=nm, name=nm)
                nc.vector.scalar_tensor_tensor(o[:], a1[:], msk1_t[0:nrows, 0:1],
                                               a0[:], ALU.mult, ALU.add)
                return o
            dl = _sel(0, DT_RANK, BF16, "dl")
            nc.vector.tensor_copy(dtlow_bf[:], dl[:])
            b_sb = _sel(DT_RANK, D_STATE, BF16, "b_sb")
            c_sb = _sel(DT_RANK + D_STATE, D_STATE, BF16, "c_sb")
            pps2 = p4.enter_context(tc.tile_pool(name="p4ps2", bufs=2, space="PSUM"))
            for c in range(NCH):
                pb = pps2.tile([128, 512], F32, tag="pb", name="pb")
                nc.tensor.matmul(pb[:], s01n_t[:], b_sb[:, 512 * c:512 * (c + 1)],
                                 start=True, stop=True)
                nc.vector.tensor_copy(brep_t[:, 512 * c:512 * (c + 1)], pb[:])
                pc = pps2.tile([128, 512], F32, tag="pc", name="pc")
                nc.tensor.matmul(pc[:], s01p_t[:], c_sb[:, 512 * c:512 * (c + 1)],
                                 start=True, stop=True)
                nc.vector.tensor_copy(crep_t[:, 512 * c:512 * (c + 1)], pc[:])

        # ============ P5: dt path ============
        with ExitStack() as p5:
            wdt = p5.enter_context(tc.tile_pool(name="wdt", bufs=1))
            w_dt_t = wdt.tile([DT_RANK, EL], BF16)
            nc.sync.dma_start(w_dt_t[:], w_dt_T)
            pps = p5.enter_context(tc.tile_pool(name="p5ps", bufs=4, space="PSUM"))
            sp = p5.enter_context(tc.tile_pool(name="p5s", bufs=2))
            sgp = p5.enter_context(tc.tile_pool(name="p5sg", bufs=1))
            # batch all Sigmoid ops, then all Ln ops: Sigmoid and Ln live in
            # different activation-function tables (1.28us reload each)
            sg_all = [sgp.tile([128, T], F32, tag=f"sg{m}", name=f"sg{m}")
                      for m in range(NET)]
            for m in range(NET):
                for c in range(NCH):
                    ps = pps.tile([128, 512], F32, tag="ps5", name="ps5")
                    nc.tensor.matmul(ps[:], w_dt_t[:, 128 * m:128 * (m + 1)],
                                     dtlow_bf[:, 512 * c:512 * (c + 1)], start=True, stop=True)
                    nc.scalar.activation(sg_all[m][:, 512 * c:512 * (c + 1)], ps[:],
                                         AF.Sigmoid, scale=-1.0, bias=dtbn_t[:, m:m + 1])
            for m in range(NET):
                nc.scalar.activation(lnsig_bf[m][:], sg_all[m][:], AF.Ln)
                dtu = sp.tile([128, T], BF16, tag="dtu", name="dtu")
                nc.vector.tensor_tensor(dtu[:], lnsig_bf[m][:], u_bf[m][:], ALU.mult)
                nc.sync.dma_start(dtu_dram[128 * m:128 * (m + 1), :], dtu[:])

        # ============ P6: scan + y-sum + gate ============
        y2p = mid.enter_context(tc.tile_pool(name="y2p", bufs=1))
        y2_bf = [y2p.tile([128, T], BF16, tag=f"y2{k}", name=f"y2{k}") for k in range(NET)]
        if ab_no_p6:
            for k in range(NET):
                nc.vector.memset(y2_bf[k][:], 0.01)
        with _skippable(), ExitStack() as p6:
            if ab_no_p6:
                raise _SkipBlock
            reps = p6.enter_context(tc.tile_pool(name="reps", bufs=2, space="PSUM"))
            yps = p6.enter_context(tc.tile_pool(name="ypsum", bufs=1, space="PSUM"))
            sp = p6.enter_context(tc.tile_pool(name="p6s", bufs=3))
            for J in range(4):
                py = yps.tile([128, T], F32, tag="py", name="py")
                for jj in range(16):
                    j = 16 * J + jj
                    dA = sp.tile([128, T], F32, tag="dA", name="dA")
                    for hf in range(2):
                        pr = reps.tile([128, 1024], F32, tag="pr", name="pr")
                        for q in range(2):
                            c = 2 * hf + q
                            nc.tensor.matmul(pr[:, 512 * q:512 * (q + 1)],
                                             r01_t[:, 128 * jj:128 * (jj + 1)],
                                             lnsig_bf[J][:, 512 * c:512 * (c + 1)],
                                             start=True, stop=True)
                        nc.scalar.activation(dA[:, 1024 * hf:1024 * (hf + 1)], pr[:],
                                             AF.Exp, scale=negA_t[:, j:j + 1])
                    dtur = sp.tile([128, T], BF16, tag="dtur", name="dtur")
                    if ab_no_dtur:
                        nc.vector.tensor_copy(dtur[:], u_bf[J][:])
                    else:
                        src = dtu_dram[128 * J + 8 * jj:128 * J + 8 * jj + 8, :]
                        nc.sync.dma_start(dtur[:], src.unsqueeze(0).broadcast_to([16, 8, T]))
                    # scan is DVE-only (walrus rejects it on Pool); the two
                    # elementwise multiplies can run on GpSimd to unload DVE
                    bb = sp.tile([128, T], BF16, tag="bb", name="bb")
                    # balance the two flexible multiplies: DVE is the scan-bound
                    # engine (2 cyc/el for scans), Pool absorbs ~1.25 of the 2
                    # elementwise passes
                    vb = nc.gpsimd if (bb_pool_mod and j % bb_pool_mod) else nc.vector
                    vb.tensor_tensor(bb[:], dtur[:], brep_t[:], ALU.mult)
                    hh = sp.tile([128, T], BF16, tag="hh", name="hh")
                    if ab_scan_tt:
                        nc.vector.tensor_tensor(hh[:], dA[:], bb[:], ALU.mult)
                    else:
                        nc.vector.tensor_tensor_scan(hh[:], dA[:], bb[:], 0.0, ALU.mult, ALU.add)
                    # C-multiply is off the scan-to-scan chain: mostly GpSimd,
                    # 1/3 back on DVE so Pool doesn't become the phase bound
                    ve = (nc.gpsimd if (j % 4) else nc.vector) if gps_mod else nc.vector
                    t1 = sp.tile([128, T], BF16, tag="t1", name="t1")
                    ve.tensor_tensor(t1[:], hh[:], crep_t[:], ALU.mult)
                    for c in range(NCH):
                        nc.tensor.matmul(py[:, 512 * c:512 * (c + 1)],
                                         g01_t[:, 128 * jj:128 * (jj + 1)],
                                         t1[:, 512 * c:512 * (c + 1)],
                                         start=(jj == 0), stop=(jj == 15))
                for c in range(NCH):
                    yd = sp.tile([128, 512], F32, tag="yd", name="yd")
                    nc.vector.scalar_tensor_tensor(yd[:], u_bf[J][:, 512 * c:512 * (c + 1)],
                                                   dcol_t[:, J:J + 1],
                                                   py[:, 512 * c:512 * (c + 1)],
                                                   ALU.mult, ALU.add)
                    nc.vector.tensor_tensor(y2_bf[J][:, 512 * c:512 * (c + 1)], yd[:],
                                            sz_bf[J][:, 512 * c:512 * (c + 1)], ALU.mult)

        # ============ P7: out_proj partial -> ReduceScatter4 ============
        with ExitStack() as p7:
            wout = p7.enter_context(tc.tile_pool(name="wout", bufs=1))
            w_out_t = [wout.tile([128, D_MODEL], BF16, tag=f"wo{k}", name=f"wo{k}") for k in range(NET)]
            for k in range(NET):
                nc.sync.dma_start(w_out_t[k][:], w_out_T[128 * k:128 * (k + 1), :])
            pps = p7.enter_context(tc.tile_pool(name="p7ps", bufs=4, space="PSUM"))
            sp = p7.enter_context(tc.tile_pool(name="p7s", bufs=4))
            for m in range(8):
                for c in range(NCH):
                    ps = pps.tile([128, 512], F32, tag="ps7", name="ps7")
                    for k in range(NET):
                        nc.tensor.matmul(ps[:], w_out_t[k][:, 128 * m:128 * (m + 1)],
                                         y2_bf[k][:, 512 * c:512 * (c + 1)],
                                         start=(k == 0), stop=(k == NET - 1))
                    ob = sp.tile([128, 512], BF16, tag="ob", name="ob")
                    # split the PSUM drain between Act and DVE (DVE is idle
                    # after the scan phase)
                    if (m + c) % 2:
                        nc.vector.tensor_copy(ob[:], ps[:])
                    else:
                        nc.scalar.copy(ob[:], ps[:])
                    rr = slice(D_MODEL * c + 128 * m, D_MODEL * c + 128 * (m + 1))
                    nc.sync.dma_start(rs_in_h[0][rr, :], ob[:, 0:TQ // 2])
                    nc.sync.dma_start(rs_in_h[1][rr, :], ob[:, TQ // 2:TQ])
            for h in range(2):
                if nocc or nocc_rs:
                    nc.sync.dma_start(rs_out_h[h], rs_in_h[h][0:D_MODEL, :])
                else:
                    nc.gpsimd.collective_compute("ReduceScatter", ALU.add,
                                                 replica_groups=g4,
                                                 ins=[rs_in_h[h]], outs=[rs_out_h[h]])

        mid.close()

        # ============ P8: MLP tail ============
        if ab_no_mlp:
            with tc.tile_pool(name="abz", bufs=1) as abz:
                zt = abz.tile([128, D_MODEL], F32)
                nc.vector.memset(zt[:], 0.0)
                for i in range(TQ // 128):
                    nc.sync.dma_start(out[128 * i:128 * (i + 1), :], zt[:])
        with _skippable(), ExitStack() as p8:
            if ab_no_mlp:
                raise _SkipBlock
            wmlp = p8.enter_context(tc.tile_pool(name="wmlp", bufs=1))
            w_fc_t = [wmlp.tile([128, 2 * D_MODEL], BF16, tag=f"wf{k}", name=f"wf{k}")
                      for k in range(8)]
            for k in range(8):
                nc.sync.dma_start(w_fc_t[k][:], w_fc_T[128 * k:128 * (k + 1), :])
            w_pr_t = [wmlp.tile([128, D_MODEL], BF16, tag=f"wp{k}", name=f"wp{k}")
                      for k in range(16)]
            for k in range(16):
                nc.sync.dma_start(w_pr_t[k][:], w_pr_T[128 * k:128 * (k + 1), :])

            ar = p8.enter_context(tc.tile_pool(name="p8a", bufs=1))
            st = p8.enter_context(tc.tile_pool(name="p8t", bufs=2))
            ppt = p8.enter_context(tc.tile_pool(name="p8pt", bufs=2, space="PSUM"))
            ppm = p8.enter_context(tc.tile_pool(name="p8pm", bufs=2, space="PSUM"))
            pp1 = p8.enter_context(tc.tile_pool(name="p8p1", bufs=1, space="PSUM"))

            TH = TQ // 2
            for th in range(2):
                t0 = TH * th
                x2_T = [ar.tile([128, TH], F32, tag=f"x2T{k}", name=f"x2T{k}")
                        for k in range(8)]
                for k in range(8):
                    nc.sync.dma_start(x2_T[k][:], xqT_dram[128 * k:128 * (k + 1), t0:t0 + TH])
                rsb = [ar.tile([128, TH], BF16, tag=f"rsb{k}", name=f"rsb{k}") for k in range(8)]
                for k in range(8):
                    nc.sync.dma_start(rsb[k][:], rs_out_h[th][128 * k:128 * (k + 1), :])
                    nc.vector.tensor_tensor(x2_T[k][:], x2_T[k][:], rsb[k][:], ALU.add)

                # rmsnorm over features via ones-matmul
                sq = [ar.tile([128, TH], BF16, tag=f"sq{k}", name=f"sq{k}") for k in range(8)]
                for k in range(8):
                    nc.scalar.activation(sq[k][:], x2_T[k][:], AF.Square)
                pss = pp1.tile([1, TH], F32, tag="pss", name="pss")
                for k in range(8):
                    nc.tensor.matmul(pss[:], ones_t[:], sq[k][:], start=(k == 0), stop=(k == 7))
                rrow = st.tile([1, TH], F32, tag="rrow", name="rrow")
                nc.scalar.activation(rrow[:], pss[:], AF.Sqrt, scale=1.0 / D_MODEL,
                                     bias=eps_t[0:1, 0:1])
                rrec = st.tile([1, TH], F32, tag="rrec", name="rrec")
                nc.vector.reciprocal(rrec[:], rrow[:])
                rbf = st.tile([1, TH], BF16, tag="rbf", name="rbf")
                nc.vector.tensor_copy(rbf[:], rrec[:])
                pr2 = pp1.tile([128, TH], F32, tag="pr2", name="pr2")
                nc.tensor.matmul(pr2[:], onesr_t[:], rbf[:], start=True, stop=True)
                x2n = [ar.tile([128, TH], BF16, tag=f"x2n{k}", name=f"x2n{k}")
                       for k in range(8)]
                for k in range(8):
                    nc.vector.tensor_tensor(x2n[k][:], x2_T[k][:], pr2[:], ALU.mult)

                # c_fc (fp8 DoubleRow) + relu^2 in bf16 — the squared path is
                # too fp8-sensitive for the error gate
                hh_t = [ar.tile([128, TH], BF16, tag=f"hh{k}", name=f"hh{k}")
                        for k in range(16)]
                for m in range(16):
                    pm = ppm.tile([128, TH], F32, tag="pmm", name="pmm")
                    for k in range(8):
                        nc.tensor.matmul(pm[:], w_fc_t[k][:, 128 * m:128 * (m + 1)],
                                         x2n[k][:], start=(k == 0), stop=(k == 7))
                    rl = st.tile([128, TH], BF16, tag="rl", name="rl")
                    nc.scalar.activation(rl[:], pm[:], AF.Relu)
                    nc.vector.tensor_tensor(hh_t[m][:], rl[:], rl[:], ALU.mult)
                # c_proj + residual
                fin = [ar.tile([128, TH], F32, tag=f"fin{k}", name=f"fin{k}") for k in range(8)]
                for m in range(8):
                    pm = ppm.tile([128, TH], F32, tag="pmm", name="pmm")
                    for k in range(16):
                        nc.tensor.matmul(pm[:], w_pr_t[k][:, 128 * m:128 * (m + 1)],
                                         hh_t[k][:], start=(k == 0), stop=(k == 15))
                    nc.vector.tensor_tensor(fin[m][:], x2_T[m][:], pm[:], ALU.add)
                # transpose to token-major + store
                for i in range(TH // 128):
                    for h in range(2):
                        pt = ppt.tile([128, 512], F32, tag="ptx", name="ptx")
                        for q in range(4):
                            m = 4 * h + q
                            nc.tensor.transpose(pt[:, 128 * q:128 * (q + 1)],
                                                fin[m][:, 128 * i:128 * (i + 1)], idf_t[:])
                        ot = st.tile([128, 512], F32, tag="ot", name="ot")
                        nc.scalar.copy(ot[:], pt[:])
                        nc.sync.dma_start(out[t0 + 128 * i:t0 + 128 * (i + 1),
                                              512 * h:512 * (h + 1)], ot[:])

    nc.compile()
    return nc


def _prep_inputs(inputs):
    x = np.asarray(inputs['x'], np.float32)
    in_proj_w = np.asarray(inputs['in_proj_w'], np.float32)
    conv_w = np.asarray(inputs['conv_w'], np.float32)
    conv_b = np.asarray(inputs['conv_b'], np.float32)
    x_proj_w = np.asarray(inputs['x_proj_w'], np.float32)
    dt_proj_w = np.asarray(inputs['dt_proj_w'], np.float32)
    dt_proj_b = np.asarray(inputs['dt_proj_b'], np.float32)
    A_log = np.asarray(inputs['A_log'], np.float32)
    D = np.asarray(inputs['D'], np.float32)
    out_proj_w = np.asarray(inputs['out_proj_w'], np.float32)
    c_fc_w = np.asarray(inputs['c_fc_w'], np.float32)
    c_proj_w = np.asarray(inputs['c_proj_w'], np.float32)

    import ml_dtypes
    bf = lambda a: np.ascontiguousarray(a).astype(ml_dtypes.bfloat16)
    f32 = lambda a: np.ascontiguousarray(a, np.float32)
    f8 = lambda a: np.ascontiguousarray(a).astype(ml_dtypes.float8_e4m3)

    r01 = np.zeros((16, 128, 128), np.float32)  # [jm][k, m] = 1 iff k == 8*jm + m%8
    g01 = np.zeros((16, 128, 128), np.float32)  # [jm][k, m] = 1 iff m == 8*jm + k%8
    for jm in range(16):
        for m in range(128):
            r01[jm, 8 * jm + (m % 8), m] = 1.0
            g01[jm, m, 8 * jm + (m % 8)] = 1.0
    s01n = np.zeros((D_STATE, 128), np.float32)
    s01p = np.zeros((D_STATE, 128), np.float32)
    for m in range(128):
        s01n[m // 8, m] = -1.0
        s01p[m // 8, m] = 1.0
    ident = np.eye(128, dtype=np.float32)

    def col_fold(a):
        # (EL,) or (EL, w) -> (128, NET*w): cols [w*k:w*(k+1)] = rows of e-tile k
        a = a.reshape(EL, -1)
        w = a.shape[1]
        o = np.zeros((128, NET * w), np.float32)
        for k in range(NET):
            o[:, w * k:w * (k + 1)] = a[128 * k:128 * (k + 1)]
        return o

    in_maps = []
    for c in range(8):
        b, r = c // 4, c % 4
        sl = slice(EL * r, EL * (r + 1))
        negA_ = np.zeros((128, NJ), np.float32)
        p = np.arange(128)
        for j in range(NJ):
            e = EL * r + 8 * j + (p % 8)
            s = p // 8
            negA_[:, j] = np.exp(A_log[e, s])
        msk0 = np.full((XD, 1), 1.0 if b == 0 else 0.0, np.float32)
        msk1 = np.full((XD, 1), 1.0 if b == 1 else 0.0, np.float32)
        cw = conv_w[sl]
        conv_d = np.zeros((128, NET * D_CONV * 128), np.float32)
        for k in range(NET):
            for tap in range(D_CONV):
                base = 128 * (D_CONV * k + tap)
                conv_d[np.arange(128), base + np.arange(128)] = cw[128 * k:128 * (k + 1), tap]
        in_maps.append({
            'xb': f32(x[b]),
            'xq': f32(x[b][TQ * r:TQ * (r + 1)]),
            'w_in_f8': f8(np.concatenate([in_proj_w[sl], in_proj_w[D_INNER:][sl]], 0).T * 256.0),
            'conv_wc': col_fold(conv_w[sl]),
            'conv_d': bf(conv_d),
            'conv_bc': col_fold(conv_b[sl]),
            'w_xp_T': bf(x_proj_w[:, sl].T),
            'w_dt_T': bf(dt_proj_w[sl].T),
            'dt_bnc': col_fold(-dt_proj_b[sl]),
            'negA': negA_,
            'd_c': col_fold(D[sl]),
            'w_out_T': bf(out_proj_w[:, sl].T),
            'w_fc_T': bf(c_fc_w.T),
            'w_pr_T': bf(c_proj_w.T),
            'r01': bf(r01.reshape(16 * 128, 128)),
            'g01': bf(g01.reshape(16 * 128, 128)),
            's01n': bf(s01n),
            's01p': bf(s01p),
            'ident_bf': bf(ident),
            'ident_f32': f32(ident),
            'ones_bf': bf(np.ones((128, 1), np.float32)),
            'ones_row_bf': bf(np.ones((1, 128), np.float32)),
            'msk0': msk0,
            'msk1': msk1,
        })
    return in_maps


def kernel(**inputs) -> np.ndarray:
    if 'nc' not in _CACHE:
        _CACHE['nc'] = _build()
    nc = _CACHE['nc']
    in_maps = _prep_inputs(inputs)
    res = run_bass_kernel_spmd(nc, in_maps, core_ids=list(range(8)))
    out = np.zeros((B, T, D_MODEL), np.float32)
    for c in range(8):
        b, r = c // 4, c % 4
        out[b, TQ * r:TQ * (r + 1), :] = res.results[c]['out']
    return out



# revision 47
# speedup vs baseline: 1.2511x; 1.1231x over previous
"""Trainium2 Bass kernel for nn_Block_4526895530469 (Mamba block + MLP residual).

Sharding over 8 NeuronCores: core c -> batch b=c//4, channel shard r=c%4
(512 of the 2048 d_inner channels), full T=2048 sequence per core. The
selective scan runs full-T per channel on the Vector engine
(tensor_tensor_scan), so there is no cross-core state chain. Layout for the
scan is (s,e)-pairs on partitions (row p = 8*s + e_local, 16 states x 8
channels per 128-row tile) x time on the free dimension.

Collectives: one zero-padded global AllReduce for the (96, T) x_dbl partial
sums (contraction over the sharded d_inner), and one 4-group ReduceScatter
for the out_proj partials which simultaneously scatters tokens for the
token-parallel MLP tail.
"""
import sys
sys.path.insert(0, '/opt/trn_rl_repo')

import numpy as np
from contextlib import ExitStack

import concourse.bass as bass
from concourse import bacc
import concourse.tile as tile
from concourse import mybir
from concourse.bass_utils import run_bass_kernel_spmd

# The interp (used by Tile's scheduling pass and by test simulation) lacks
# Silu; emulate it: run the existing Sigmoid path, then multiply by the
# scaled/biased input.
from concourse import bass_interp as _bi
from concourse import mybir as _mb

_orig_visit_act = _bi.InstructionExecutor.visit_InstActivation


def _visit_act_with_silu(self, instruction, *a, **kw):
    if instruction.func != _mb.ActivationFunctionType.Silu:
        return _orig_visit_act(self, instruction, *a, **kw)
    import numpy as _np
    assert len(instruction.outs) == 1, "Silu shim: no accum_out support"
    func0 = instruction.func
    try:
        instruction.func = _mb.ActivationFunctionType.Sigmoid
        res = _orig_visit_act(self, instruction, *a, **kw)
    finally:
        instruction.func = func0
    reg_snapshot = kw.get("reg_snapshot")
    inp = self.view_ap(instruction.ins[0], _bi.Direction.READ, instruction,
                       reg_snapshot=reg_snapshot).astype(_np.float32)
    inp = inp.reshape(inp.shape[0], -1)

    def _val(arg):
        if isinstance(arg, _mb.ImmediateValue):
            return arg.value
        v = self.view_ap(arg, _bi.Direction.READ, instruction,
                         reg_snapshot=reg_snapshot).astype(_np.float32)
        return v.reshape(v.shape[0], -1)

    bias = _val(instruction.ins[1])
    scale = _val(instruction.ins[2])
    sx = inp * scale + bias
    out_view = self.view_ap(instruction.outs[0], _bi.Direction.WRITE, instruction,
                            reg_snapshot=reg_snapshot)
    sig = _np.asarray(out_view, dtype=_np.float32).reshape(sx.shape)
    out_view[:] = (sig * sx).reshape(out_view.shape).astype(out_view.dtype)
    return res


_bi.InstructionExecutor.visit_InstActivation = _visit_act_with_silu

F32 = mybir.dt.float32
BF16 = mybir.dt.bfloat16
FP8 = mybir.dt.float8e4
AF = mybir.ActivationFunctionType
ALU = mybir.AluOpType

D_MODEL, D_INNER, D_STATE, D_CONV, DT_RANK = 1024, 2048, 16, 4, 64
B, T = 2, 2048
EL = D_INNER // 4          # 512 channels per core
NET = EL // 128            # 4 e-tiles
NJ = EL // 8               # 64 scan tiles
NCH = T // 512             # 4 t-chunks
TQ = T // 4                # 512 tokens for the MLP tail
XD = DT_RANK + 2 * D_STATE  # 96
EPS = float(np.finfo(np.float32).eps)

_CACHE = {}


class _SkipBlock(Exception):
    pass


from contextlib import contextmanager


@contextmanager
def _skippable():
    try:
        yield
    except _SkipBlock:
        pass


def _build(nocc=False, gps_mod=2, ar_bf16=True, nocc_ar=False, nocc_rs=False, rs_split=True,
           ab_no_p6=False, ab_scan_tt=False, ab_no_dtur=False, ab_no_p1p2=False,
           ab_no_mlp=False, bb_pool_mod=2):
    nc = bacc.Bacc("TRN2", target_bir_lowering=False, debug=False, num_devices=8)

    def din(name, shape, dt=BF16):
        return nc.dram_tensor(name, list(shape), dt, kind="ExternalInput").ap()

    xb = din("xb", (T, D_MODEL), F32)
    xq = din("xq", (TQ, D_MODEL), F32)
    w_in_f8 = din("w_in_f8", (D_MODEL, 2 * EL), FP8)   # pre-scaled x256
    conv_wc = din("conv_wc", (128, NET * D_CONV), F32)   # cols [4k:4k+4] = e-tile k
    conv_d = din("conv_d", (128, NET * D_CONV * 128))    # [128,128] diag blocks
    conv_bc = din("conv_bc", (128, NET), F32)
    w_xp_T = din("w_xp_T", (EL, XD))
    w_dt_T = din("w_dt_T", (DT_RANK, EL))
    dt_bnc = din("dt_bnc", (128, NET), F32)              # -dt_proj_b
    negA = din("negA", (128, NJ), F32)                   # exp(A_log) per scan tile col
    d_c = din("d_c", (128, NET), F32)
    w_out_f8 = din("w_out_f8", (EL, D_MODEL), FP8)   # x256
    w_fc_T = din("w_fc_T", (D_MODEL, 2 * D_MODEL))
    w_pr_T = din("w_pr_T", (2 * D_MODEL, D_MODEL))
    r01 = din("r01", (16 * 128, 128))
    g01 = din("g01", (16 * 128, 128))
    s01n = din("s01n", (D_STATE, 128))
    s01p = din("s01p", (D_STATE, 128))
    ident_bf = din("ident_bf", (128, 128))
    ident_f32 = din("ident_f32", (128, 128), F32)
    ones_bf = din("ones_bf", (128, 1))
    ones_row_bf = din("ones_row_bf", (1, 128))
    msk0 = din("msk0", (XD, 1), F32)
    msk1 = din("msk1", (XD, 1), F32)

    out = nc.dram_tensor("out", [TQ, D_MODEL], F32, kind="ExternalOutput").ap()

    xdbl_in = nc.dram_tensor("xdbl_in", [2 * XD, T], BF16).ap()
    xdbl_out = nc.dram_tensor("xdbl_out", [2 * XD, T], BF16).ap()
    rs_in_h = [nc.dram_tensor(f"rs_in{h}", [4 * D_MODEL, TQ // 2], BF16).ap() for h in range(2)]
    rs_out_h = [nc.dram_tensor(f"rs_out{h}", [D_MODEL, TQ // 2], BF16).ap() for h in range(2)]
    dtu_dram = nc.dram_tensor("dtu_dram", [EL, T], BF16).ap()
    xqT_dram = nc.dram_tensor("xqT_dram", [D_MODEL, TQ], F32).ap()

    g8 = [[0, 1, 2, 3, 4, 5, 6, 7]]
    g4 = [[0, 1, 2, 3], [4, 5, 6, 7]]

    with tile.TileContext(nc) as tc, ExitStack() as top:
        cpool = top.enter_context(tc.tile_pool(name="consts", bufs=1))

        def cload(nm, name_ap, shape, dt=BF16):
            t = cpool.tile(list(shape), dt, tag=nm, name=nm)
            nc.sync.dma_start(t[:], name_ap)
            return t

        negA_t = cload("negA_t", negA, (128, NJ), F32)
        convd_t = cload("convd_t", conv_d, (128, NET * D_CONV * 128))
        convb_t = cload("convb_t", conv_bc, (128, NET), F32)
        dtbn_t = cload("dtbn_t", dt_bnc, (128, NET), F32)
        dcol_t = cload("dcol_t", d_c, (128, NET), F32)
        s01n_t = cload("s01n_t", s01n, (D_STATE, 128))
        s01p_t = cload("s01p_t", s01p, (D_STATE, 128))
        idb_t = cload("idb_t", ident_bf, (128, 128))
        idf_t = cload("idf_t", ident_f32, (128, 128), F32)
        ones_t = cload("ones_t", ones_bf, (128, 1))
        onesr_t = cload("onesr_t", ones_row_bf, (1, 128))
        msk0_t = cload("msk0_t", msk0, (XD, 1), F32)
        msk1_t = cload("msk1_t", msk1, (XD, 1), F32)
        eps_t = cpool.tile([128, 1], F32)
        nc.vector.memset(eps_t[:], EPS)
        eps256_t = cpool.tile([128, 1], F32)
        nc.vector.memset(eps256_t[:], EPS / 256.0)
        c16_t = cpool.tile([128, 1], F32)
        nc.vector.memset(c16_t[:], 16.0)
        cinv4k_t = cpool.tile([128, 1], F32)
        nc.vector.memset(cinv4k_t[:], 1.0 / 4096.0)
        r01_t = cpool.tile([128, 16 * 128], BF16)
        for k in range(16):
            nc.sync.dma_start(r01_t[:, 128 * k:128 * (k + 1)], r01[128 * k:128 * (k + 1), :])
        g01_t = cpool.tile([128, 16 * 128], BF16)
        for k in range(16):
            nc.sync.dma_start(g01_t[:, 128 * k:128 * (k + 1)], g01[128 * k:128 * (k + 1), :])

        # long-lived activations through the scan phase (freed before MLP)
        mid = top.enter_context(ExitStack())
        acts = mid.enter_context(tc.tile_pool(name="acts", bufs=1))
        u_bf = [acts.tile([128, T], BF16, tag=f"u{k}", name=f"u{k}") for k in range(NET)]
        sz_bf = [acts.tile([128, T], BF16, tag=f"sz{k}", name=f"sz{k}") for k in range(NET)]
        lnsig_bf = [acts.tile([128, T], BF16, tag=f"lns{k}", name=f"lns{k}") for k in range(NET)]
        brep_t = acts.tile([128, T], BF16, tag="brep", name="brep")
        crep_t = acts.tile([128, T], BF16, tag="crep", name="crep")
        dtlow_bf = acts.tile([DT_RANK, T], BF16, tag="dtlow", name="dtlow")

        # ============ P1+P2: rmsnorm, transpose, in_proj ============
        with ExitStack() as ph:
            winp = ph.enter_context(tc.tile_pool(name="win", bufs=1))
            # fp8 DoubleRow layout: k-tiles on a free axis so one AP can
            # address a (2k, 2k+1) pair for the double-pumped matmul
            w_in_t = winp.tile([128, 8, 2 * EL], FP8, tag="wi8", name="wi8")
            for k in range(8):
                nc.sync.dma_start(w_in_t[:, k, :], w_in_f8[128 * k:128 * (k + 1), :])

            xinp = ph.enter_context(tc.tile_pool(name="xinz", bufs=1))
            # 3 zero pad columns in front so the causal-conv PE matmuls can
            # read shifted windows without bounds checks
            x_in = [xinp.tile([128, 3 + T], BF16, tag=f"xin{k}", name=f"xin{k}")
                    for k in range(NET)]
            for k in range(NET):
                nc.vector.memset(x_in[k][:, 0:3], 0.0)

            if ab_no_p1p2:
                for k in range(NET):
                    nc.vector.memset(x_in[k][:], 0.01)
                    nc.vector.memset(sz_bf[k][:], 0.01)
            with _skippable(), ExitStack() as p1:
                if ab_no_p1p2:
                    raise _SkipBlock
                xnp = p1.enter_context(tc.tile_pool(name="xnT", bufs=1))
                xn8 = xnp.tile([128, 8, T], FP8, tag="xn8", name="xn8")
                str_p = p1.enter_context(tc.tile_pool(name="p1s", bufs=3))
                xtmp = p1.enter_context(tc.tile_pool(name="p1x", bufs=6))
                jp = p1.enter_context(tc.tile_pool(name="p1j", bufs=1))
                junk = jp.tile([128, D_MODEL], BF16)
                pps1 = p1.enter_context(tc.tile_pool(name="p1ps", bufs=2, space="PSUM"))

                for c in range(NCH):
                    xnt4 = []
                    for q in range(4):
                        i = 4 * c + q
                        xt = str_p.tile([128, D_MODEL], BF16, tag="xt", name="xt")
                        nc.gpsimd.dma_start(xt[:], xb[128 * i:128 * (i + 1), :])
                        ssq = str_p.tile([128, 1], F32, tag="ssq", name="ssq")
                        nc.scalar.activation(junk[:], xt[:], AF.Square, accum_out=ssq[:])
                        rr = str_p.tile([128, 1], F32, tag="rr", name="rr")
                        nc.scalar.activation(rr[:], ssq[:], AF.Sqrt, scale=1.0 / D_MODEL,
                                             bias=eps_t[:, 0:1])
                        rc = str_p.tile([128, 1], F32, tag="rc", name="rc")
                        nc.vector.reciprocal(rc[:], rr[:])
                        xnt = xtmp.tile([128, D_MODEL], BF16, tag="xnt", name="xnt")
                        nc.vector.tensor_scalar_mul(xnt[:], xt[:], rc[:, 0:1])
                        xnt4.append(xnt)
                    for k in range(8):
                        pt = pps1.tile([128, 512], BF16, tag="pt", name="pt")
                        for q in range(4):
                            nc.tensor.transpose(pt[:, 128 * q:128 * (q + 1)],
                                                xnt4[q][:, 128 * k:128 * (k + 1)],
                                                idb_t[:])
                        # cast to fp8 with a x16 range shift (fp8e4 min-normal
                        # is 2^-6; unscaled unit-RMS values lose mantissa);
                        # alternate engines to balance Act vs DVE
                        if (k + c) % 2:
                            nc.vector.tensor_scalar_mul(xn8[:, k, 512 * c:512 * (c + 1)],
                                                        pt[:], c16_t[:, 0:1])
                        else:
                            nc.scalar.activation(xn8[:, k, 512 * c:512 * (c + 1)], pt[:],
                                                 AF.Copy, scale=16.0)

                # in_proj: fp8 DoubleRow (two 128-deep k-tiles per matmul)
                pps2 = p1.enter_context(tc.tile_pool(name="p2ps", bufs=4, space="PSUM"))
                UNSC = 1.0 / (256.0 * 16.0)
                for m in range(8):
                    for c in range(NCH):
                        ps = pps2.tile([128, 512], F32, tag="ps", name="ps")
                        for p2 in range(4):
                            nc.tensor.matmul(ps[:], w_in_t[:, 2 * p2:2 * p2 + 2,
                                                          128 * m:128 * (m + 1)],
                                             xn8[:, 2 * p2:2 * p2 + 2,
                                                 512 * c:512 * (c + 1)],
                                             start=(p2 == 0), stop=(p2 == 3),
                                             perf_mode=mybir.MatmulPerfMode.DoubleRow)
                        if m < 4:
                            nc.vector.tensor_scalar_mul(x_in[m][:, 3 + 512 * c:3 + 512 * (c + 1)],
                                                        ps[:], cinv4k_t[:, 0:1])
                        else:
                            nc.scalar.activation(sz_bf[m - 4][:, 512 * c:512 * (c + 1)],
                                                 ps[:], AF.Silu, scale=UNSC)

            # ============ P3: conv (PE diag-matmuls) + silu -> u ============
            with ExitStack() as p3:
                pps3 = p3.enter_context(tc.tile_pool(name="p3ps", bufs=2, space="PSUM"))
                for k in range(NET):
                    for c in range(NCH):
                        ps = pps3.tile([128, 512], F32, tag="psc", name="psc")
                        for tap in range(D_CONV):
                            sh = D_CONV - 1 - tap       # time shift for this tap
                            dcol = 128 * (D_CONV * k + tap)
                            nc.tensor.matmul(ps[:], convd_t[:, dcol:dcol + 128],
                                             x_in[k][:, 3 + 512 * c - sh:3 + 512 * (c + 1) - sh],
                                             start=(tap == 0), stop=(tap == D_CONV - 1))
                        nc.scalar.activation(u_bf[k][:, 512 * c:512 * (c + 1)], ps[:],
                                             AF.Silu, bias=convb_t[:, k:k + 1])

        # ============ P4: x_proj partial -> AllReduce8 -> dtlow/B_rep/C_rep ============
        with ExitStack() as p4:
            wxp = p4.enter_context(tc.tile_pool(name="wxp", bufs=1))
            w_xp_t = [wxp.tile([128, XD], BF16, tag=f"wxp{k}", name=f"wxp{k}") for k in range(NET)]
            for k in range(NET):
                nc.sync.dma_start(w_xp_t[k][:], w_xp_T[128 * k:128 * (k + 1), :])
            pps = p4.enter_context(tc.tile_pool(name="p4ps", bufs=2, space="PSUM"))
            sp = p4.enter_context(tc.tile_pool(name="p4s", bufs=2))
            big = p4.enter_context(tc.tile_pool(name="p4big", bufs=1))
            for c in range(NCH):
                ps = pps.tile([XD, 512], F32, tag="ps4", name="ps4")
                for k in range(NET):
                    nc.tensor.matmul(ps[:], w_xp_t[k][:], u_bf[k][:, 512 * c:512 * (c + 1)],
                                     start=(k == 0), stop=(k == NET - 1))
                t0 = sp.tile([XD, 512], BF16, tag="t0", name="t0")
                nc.vector.tensor_scalar_mul(t0[:], ps[:], msk0_t[:, 0:1])
                t1 = sp.tile([XD, 512], BF16, tag="t1", name="t1")
                nc.vector.tensor_scalar_mul(t1[:], ps[:], msk1_t[:, 0:1])
                nc.sync.dma_start(xdbl_in[0:XD, 512 * c:512 * (c + 1)], t0[:])
                nc.sync.dma_start(xdbl_in[XD:2 * XD, 512 * c:512 * (c + 1)], t1[:])
            if nocc or nocc_ar:
                nc.sync.dma_start(xdbl_out, xdbl_in)
            else:
                nc.gpsimd.collective_compute("AllReduce", ALU.add, replica_groups=g8,
                                             ins=[xdbl_in], outs=[xdbl_out])
            # fill the AllReduce window: transpose the residual token-quarter
            # (f32) now and stage it in DRAM for the MLP tail
            ppq = p4.enter_context(tc.tile_pool(name="p4q", bufs=2, space="PSUM"))
            spq = p4.enter_context(tc.tile_pool(name="p4qs", bufs=2))
            for i in range(TQ // 128):
                xt_ = spq.tile([128, D_MODEL], F32, tag="xq_tm", name="xq_tm")
                nc.sync.dma_start(xt_[:], xq[128 * i:128 * (i + 1), :])
                for h in range(2):
                    ptq = ppq.tile([128, 512], F32, tag="ptq", name="ptq")
                    for q in range(4):
                        k = 4 * h + q
                        nc.tensor.transpose(ptq[:, 128 * q:128 * (q + 1)],
                                            xt_[:, 128 * k:128 * (k + 1)], idf_t[:])
                    otq = spq.tile([128, 512], F32, tag="otq", name="otq")
                    nc.scalar.copy(otq[:], ptq[:])
                    for q in range(4):
                        k = 4 * h + q
                        nc.sync.dma_start(
                            xqT_dram[128 * k:128 * (k + 1), 128 * i:128 * (i + 1)],
                            otq[:, 128 * q:128 * (q + 1)])
            # batch-select each 32-aligned section separately (compute ops
            # cannot start at partition 80)
            def _sel(rows, nrows, out_dt, nm):
                a0 = big.tile([nrows, T], BF16, tag=nm + "a0", name=nm + "a0")
                nc.sync.dma_start(a0[:], xdbl_out[rows:rows + nrows, :])
                a1 = big.tile([nrows, T], BF16, tag=nm + "a1", name=nm + "a1")
                nc.sync.dma_start(a1[:], xdbl_out[XD + rows:XD + rows + nrows, :])
                nc.vector.tensor_scalar_mul(a0[:], a0[:], msk0_t[0:nrows, 0:1])
                o = big.tile([nrows, T], out_dt, tag=nm, name=nm)
                nc.vector.scalar_tensor_tensor(o[:], a1[:], msk1_t[0:nrows, 0:1],
                                               a0[:], ALU.mult, ALU.add)
                return o
            dl = _sel(0, DT_RANK, BF16, "dl")
            nc.vector.tensor_copy(dtlow_bf[:], dl[:])
            b_sb = _sel(DT_RANK, D_STATE, BF16, "b_sb")
            c_sb = _sel(DT_RANK + D_STATE, D_STATE, BF16, "c_sb")
            pps2 = p4.enter_context(tc.tile_pool(name="p4ps2", bufs=2, space="PSUM"))
            for c in range(NCH):
                pb = pps2.tile([128, 512], F32, tag="pb", name="pb")
                nc.tensor.matmul(pb[:], s01n_t[:], b_sb[:, 512 * c:512 * (c + 1)],
                                 start=True, stop=True)
                nc.vector.tensor_copy(brep_t[:, 512 * c:512 * (c + 1)], pb[:])
                pc = pps2.tile([128, 512], F32, tag="pc", name="pc")
                nc.tensor.matmul(pc[:], s01p_t[:], c_sb[:, 512 * c:512 * (c + 1)],
                                 start=True, stop=True)
                nc.vector.tensor_copy(crep_t[:, 512 * c:512 * (c + 1)], pc[:])

        # ============ P5: dt path ============
        with ExitStack() as p5:
            wdt = p5.enter_context(tc.tile_pool(name="wdt", bufs=1))
            w_dt_t = wdt.tile([DT_RANK, EL], BF16)
            nc.sync.dma_start(w_dt_t[:], w_dt_T)
            pps = p5.enter_context(tc.tile_pool(name="p5ps", bufs=4, space="PSUM"))
            sp = p5.enter_context(tc.tile_pool(name="p5s", bufs=2))
            sgp = p5.enter_context(tc.tile_pool(name="p5sg", bufs=1))
            # batch all Sigmoid ops, then all Ln ops: Sigmoid and Ln live in
            # different activation-function tables (1.28us reload each)
            sg_all = [sgp.tile([128, T], F32, tag=f"sg{m}", name=f"sg{m}")
                      for m in range(NET)]
            for m in range(NET):
                for c in range(NCH):
                    ps = pps.tile([128, 512], F32, tag="ps5", name="ps5")
                    nc.tensor.matmul(ps[:], w_dt_t[:, 128 * m:128 * (m + 1)],
                                     dtlow_bf[:, 512 * c:512 * (c + 1)], start=True, stop=True)
                    nc.scalar.activation(sg_all[m][:, 512 * c:512 * (c + 1)], ps[:],
                                         AF.Sigmoid, scale=-1.0, bias=dtbn_t[:, m:m + 1])
            for m in range(NET):
                nc.scalar.activation(lnsig_bf[m][:], sg_all[m][:], AF.Ln)
                dtu = sp.tile([128, T], BF16, tag="dtu", name="dtu")
                nc.vector.tensor_tensor(dtu[:], lnsig_bf[m][:], u_bf[m][:], ALU.mult)
                nc.sync.dma_start(dtu_dram[128 * m:128 * (m + 1), :], dtu[:])

        # ============ P6: scan + y-sum + gate ============
        y2p = mid.enter_context(tc.tile_pool(name="y2p", bufs=1))
        y2f8 = y2p.tile([128, NET, T], FP8, tag="y2f8", name="y2f8")
        if ab_no_p6:
            nc.vector.memset(y2f8[:], 0.01)
        with _skippable(), ExitStack() as p6:
            if ab_no_p6:
                raise _SkipBlock
            reps = p6.enter_context(tc.tile_pool(name="reps", bufs=2, space="PSUM"))
            yps = p6.enter_context(tc.tile_pool(name="ypsum", bufs=1, space="PSUM"))
            sp = p6.enter_context(tc.tile_pool(name="p6s", bufs=3))
            for J in range(4):
                py = yps.tile([128, T], F32, tag="py", name="py")
                for jj in range(16):
                    j = 16 * J + jj
                    dA = sp.tile([128, T], F32, tag="dA", name="dA")
                    for hf in range(2):
                        pr = reps.tile([128, 1024], F32, tag="pr", name="pr")
                        for q in range(2):
                            c = 2 * hf + q
                            nc.tensor.matmul(pr[:, 512 * q:512 * (q + 1)],
                                             r01_t[:, 128 * jj:128 * (jj + 1)],
                                             lnsig_bf[J][:, 512 * c:512 * (c + 1)],
                                             start=True, stop=True)
                        nc.scalar.activation(dA[:, 1024 * hf:1024 * (hf + 1)], pr[:],
                                             AF.Exp, scale=negA_t[:, j:j + 1])
                    dtur = sp.tile([128, T], BF16, tag="dtur", name="dtur")
                    if ab_no_dtur:
                        nc.vector.tensor_copy(dtur[:], u_bf[J][:])
                    else:
                        src = dtu_dram[128 * J + 8 * jj:128 * J + 8 * jj + 8, :]
                        nc.sync.dma_start(dtur[:], src.unsqueeze(0).broadcast_to([16, 8, T]))
                    # scan is DVE-only (walrus rejects it on Pool); the two
                    # elementwise multiplies can run on GpSimd to unload DVE
                    bb = sp.tile([128, T], BF16, tag="bb", name="bb")
                    # balance the two flexible multiplies: DVE is the scan-bound
                    # engine (2 cyc/el for scans), Pool absorbs ~1.25 of the 2
                    # elementwise passes
                    vb = nc.gpsimd if (bb_pool_mod and j % bb_pool_mod) else nc.vector
                    vb.tensor_tensor(bb[:], dtur[:], brep_t[:], ALU.mult)
                    hh = sp.tile([128, T], BF16, tag="hh", name="hh")
                    if ab_scan_tt:
                        nc.vector.tensor_tensor(hh[:], dA[:], bb[:], ALU.mult)
                    else:
                        nc.vector.tensor_tensor_scan(hh[:], dA[:], bb[:], 0.0, ALU.mult, ALU.add)
                    # C-multiply is off the scan-to-scan chain: mostly GpSimd,
                    # 1/3 back on DVE so Pool doesn't become the phase bound
                    ve = (nc.gpsimd if (j % 4) else nc.vector) if gps_mod else nc.vector
                    t1 = sp.tile([128, T], BF16, tag="t1", name="t1")
                    ve.tensor_tensor(t1[:], hh[:], crep_t[:], ALU.mult)
                    for c in range(NCH):
                        nc.tensor.matmul(py[:, 512 * c:512 * (c + 1)],
                                         g01_t[:, 128 * jj:128 * (jj + 1)],
                                         t1[:, 512 * c:512 * (c + 1)],
                                         start=(jj == 0), stop=(jj == 15))
                for c in range(NCH):
                    yd = sp.tile([128, 512], F32, tag="yd", name="yd")
                    nc.vector.scalar_tensor_tensor(yd[:], u_bf[J][:, 512 * c:512 * (c + 1)],
                                                   dcol_t[:, J:J + 1],
                                                   py[:, 512 * c:512 * (c + 1)],
                                                   ALU.mult, ALU.add)
                    nc.vector.tensor_tensor(y2f8[:, J, 512 * c:512 * (c + 1)], yd[:],
                                            sz_bf[J][:, 512 * c:512 * (c + 1)], ALU.mult)

        # ============ P7: out_proj partial -> ReduceScatter4 ============
        with ExitStack() as p7:
            wout = p7.enter_context(tc.tile_pool(name="wout", bufs=1))
            w_out_t = wout.tile([128, NET, D_MODEL], FP8, tag="wo8", name="wo8")
            for k in range(NET):
                nc.sync.dma_start(w_out_t[:, k, :], w_out_f8[128 * k:128 * (k + 1), :])
            unso_t = wout.tile([128, 1], F32, tag="unso", name="unso")
            nc.vector.memset(unso_t[:], 1.0 / 2048.0)
            pps = p7.enter_context(tc.tile_pool(name="p7ps", bufs=4, space="PSUM"))
            sp = p7.enter_context(tc.tile_pool(name="p7s", bufs=4))
            for m in range(8):
                for c in range(NCH):
                    ps = pps.tile([128, 512], F32, tag="ps7", name="ps7")
                    for p2 in range(NET // 2):
                        nc.tensor.matmul(ps[:], w_out_t[:, 2 * p2:2 * p2 + 2,
                                                        128 * m:128 * (m + 1)],
                                         y2f8[:, 2 * p2:2 * p2 + 2,
                                              512 * c:512 * (c + 1)],
                                         start=(p2 == 0), stop=(p2 == 1),
                                         perf_mode=mybir.MatmulPerfMode.DoubleRow)
                    ob = sp.tile([128, 512], BF16, tag="ob", name="ob")
                    # split the PSUM drain between Act and DVE (DVE is idle
                    # after the scan phase); 1/2048 undoes the fp8 w x256 and
                    # y2 x8 range shifts
                    if (m + c) % 2:
                        nc.vector.tensor_scalar_mul(ob[:], ps[:], unso_t[:, 0:1])
                    else:
                        nc.scalar.activation(ob[:], ps[:], AF.Copy, scale=1.0 / 2048.0)
                    rr = slice(D_MODEL * c + 128 * m, D_MODEL * c + 128 * (m + 1))
                    nc.sync.dma_start(rs_in_h[0][rr, :], ob[:, 0:TQ // 2])
                    nc.sync.dma_start(rs_in_h[1][rr, :], ob[:, TQ // 2:TQ])
            for h in range(2):
                if nocc or nocc_rs:
                    nc.sync.dma_start(rs_out_h[h], rs_in_h[h][0:D_MODEL, :])
                else:
                    nc.gpsimd.collective_compute("ReduceScatter", ALU.add,
                                                 replica_groups=g4,
                                                 ins=[rs_in_h[h]], outs=[rs_out_h[h]])

        mid.close()

        # ============ P8: MLP tail ============
        if ab_no_mlp:
            with tc.tile_pool(name="abz", bufs=1) as abz:
                zt = abz.tile([128, D_MODEL], F32)
                nc.vector.memset(zt[:], 0.0)
                for i in range(TQ // 128):
                    nc.sync.dma_start(out[128 * i:128 * (i + 1), :], zt[:])
        with _skippable(), ExitStack() as p8:
            if ab_no_mlp:
                raise _SkipBlock
            wmlp = p8.enter_context(tc.tile_pool(name="wmlp", bufs=1))
            w_fc_t = [wmlp.tile([128, 2 * D_MODEL], BF16, tag=f"wf{k}", name=f"wf{k}")
                      for k in range(8)]
            for k in range(8):
                nc.sync.dma_start(w_fc_t[k][:], w_fc_T[128 * k:128 * (k + 1), :])
            w_pr_t = [wmlp.tile([128, D_MODEL], BF16, tag=f"wp{k}", name=f"wp{k}")
                      for k in range(16)]
            for k in range(16):
                nc.sync.dma_start(w_pr_t[k][:], w_pr_T[128 * k:128 * (k + 1), :])

            ar = p8.enter_context(tc.tile_pool(name="p8a", bufs=1))
            st = p8.enter_context(tc.tile_pool(name="p8t", bufs=2))
            ppt = p8.enter_context(tc.tile_pool(name="p8pt", bufs=2, space="PSUM"))
            ppm = p8.enter_context(tc.tile_pool(name="p8pm", bufs=2, space="PSUM"))
            pp1 = p8.enter_context(tc.tile_pool(name="p8p1", bufs=1, space="PSUM"))

            TH = TQ // 2
            for th in range(2):
                t0 = TH * th
                x2_T = [ar.tile([128, TH], F32, tag=f"x2T{k}", name=f"x2T{k}")
                        for k in range(8)]
                for k in range(8):
                    nc.sync.dma_start(x2_T[k][:], xqT_dram[128 * k:128 * (k + 1), t0:t0 + TH])
                rsb = [ar.tile([128, TH], BF16, tag=f"rsb{k}", name=f"rsb{k}") for k in range(8)]
                for k in range(8):
                    nc.sync.dma_start(rsb[k][:], rs_out_h[th][128 * k:128 * (k + 1), :])
                    nc.vector.tensor_tensor(x2_T[k][:], x2_T[k][:], rsb[k][:], ALU.add)

                # rmsnorm over features via ones-matmul
                sq = [ar.tile([128, TH], BF16, tag=f"sq{k}", name=f"sq{k}") for k in range(8)]
                for k in range(8):
                    nc.scalar.activation(sq[k][:], x2_T[k][:], AF.Square)
                pss = pp1.tile([1, TH], F32, tag="pss", name="pss")
                for k in range(8):
                    nc.tensor.matmul(pss[:], ones_t[:], sq[k][:], start=(k == 0), stop=(k == 7))
                rrow = st.tile([1, TH], F32, tag="rrow", name="rrow")
                nc.scalar.activation(rrow[:], pss[:], AF.Sqrt, scale=1.0 / D_MODEL,
                                     bias=eps_t[0:1, 0:1])
                rrec = st.tile([1, TH], F32, tag="rrec", name="rrec")
                nc.vector.reciprocal(rrec[:], rrow[:])
                rbf = st.tile([1, TH], BF16, tag="rbf", name="rbf")
                nc.vector.tensor_copy(rbf[:], rrec[:])
                pr2 = pp1.tile([128, TH], F32, tag="pr2", name="pr2")
                nc.tensor.matmul(pr2[:], onesr_t[:], rbf[:], start=True, stop=True)
                x2n = [ar.tile([128, TH], BF16, tag=f"x2n{k}", name=f"x2n{k}")
                       for k in range(8)]
                for k in range(8):
                    nc.vector.tensor_tensor(x2n[k][:], x2_T[k][:], pr2[:], ALU.mult)

                # c_fc (fp8 DoubleRow) + relu^2 in bf16 — the squared path is
                # too fp8-sensitive for the error gate
                hh_t = [ar.tile([128, TH], BF16, tag=f"hh{k}", name=f"hh{k}")
                        for k in range(16)]
                for m in range(16):
                    pm = ppm.tile([128, TH], F32, tag="pmm", name="pmm")
                    for k in range(8):
                        nc.tensor.matmul(pm[:], w_fc_t[k][:, 128 * m:128 * (m + 1)],
                                         x2n[k][:], start=(k == 0), stop=(k == 7))
                    rl = st.tile([128, TH], BF16, tag="rl", name="rl")
                    nc.scalar.activation(rl[:], pm[:], AF.Relu)
                    nc.vector.tensor_tensor(hh_t[m][:], rl[:], rl[:], ALU.mult)
                # c_proj + residual
                fin = [ar.tile([128, TH], F32, tag=f"fin{k}", name=f"fin{k}") for k in range(8)]
                for m in range(8):
                    pm = ppm.tile([128, TH], F32, tag="pmm", name="pmm")
                    for k in range(16):
                        nc.tensor.matmul(pm[:], w_pr_t[k][:, 128 * m:128 * (m + 1)],
                                         hh_t[k][:], start=(k == 0), stop=(k == 15))
                    nc.vector.tensor_tensor(fin[m][:], x2_T[m][:], pm[:], ALU.add)
                # transpose to token-major + store
                for i in range(TH // 128):
                    for h in range(2):
                        pt = ppt.tile([128, 512], F32, tag="ptx", name="ptx")
                        for q in range(4):
                            m = 4 * h + q
                            nc.tensor.transpose(pt[:, 128 * q:128 * (q + 1)],
                                                fin[m][:, 128 * i:128 * (i + 1)], idf_t[:])
                        ot = st.tile([128, 512], F32, tag="ot", name="ot")
                        nc.scalar.copy(ot[:], pt[:])
                        nc.sync.dma_start(out[t0 + 128 * i:t0 + 128 * (i + 1),
                                              512 * h:512 * (h + 1)], ot[:])

    nc.compile()
    return nc


def _prep_inputs(inputs):
    x = np.asarray(inputs['x'], np.float32)
    in_proj_w = np.asarray(inputs['in_proj_w'], np.float32)
    conv_w = np.asarray(inputs['conv_w'], np.float32)
    conv_b = np.asarray(inputs['conv_b'], np.float32)
    x_proj_w = np.asarray(inputs['x_proj_w'], np.float32)
    dt_proj_w = np.asarray(inputs['dt_proj_w'], np.float32)
    dt_proj_b = np.asarray(inputs['dt_proj_b'], np.float32)
    A_log = np.asarray(inputs['A_log'], np.float32)
    D = np.asarray(inputs['D'], np.float32)
    out_proj_w = np.asarray(inputs['out_proj_w'], np.float32)
    c_fc_w = np.asarray(inputs['c_fc_w'], np.float32)
    c_proj_w = np.asarray(inputs['c_proj_w'], np.float32)

    import ml_dtypes
    bf = lambda a: np.ascontiguousarray(a).astype(ml_dtypes.bfloat16)
    f32 = lambda a: np.ascontiguousarray(a, np.float32)
    f8 = lambda a: np.ascontiguousarray(a).astype(ml_dtypes.float8_e4m3)

    r01 = np.zeros((16, 128, 128), np.float32)  # [jm][k, m] = 1 iff k == 8*jm + m%8
    g01 = np.zeros((16, 128, 128), np.float32)  # [jm][k, m] = 1 iff m == 8*jm + k%8
    for jm in range(16):
        for m in range(128):
            r01[jm, 8 * jm + (m % 8), m] = 1.0
            g01[jm, m, 8 * jm + (m % 8)] = 1.0
    s01n = np.zeros((D_STATE, 128), np.float32)
    s01p = np.zeros((D_STATE, 128), np.float32)
    for m in range(128):
        s01n[m // 8, m] = -1.0
        s01p[m // 8, m] = 1.0
    ident = np.eye(128, dtype=np.float32)

    def col_fold(a):
        # (EL,) or (EL, w) -> (128, NET*w): cols [w*k:w*(k+1)] = rows of e-tile k
        a = a.reshape(EL, -1)
        w = a.shape[1]
        o = np.zeros((128, NET * w), np.float32)
        for k in range(NET):
            o[:, w * k:w * (k + 1)] = a[128 * k:128 * (k + 1)]
        return o

    in_maps = []
    for c in range(8):
        b, r = c // 4, c % 4
        sl = slice(EL * r, EL * (r + 1))
        negA_ = np.zeros((128, NJ), np.float32)
        p = np.arange(128)
        for j in range(NJ):
            e = EL * r + 8 * j + (p % 8)
            s = p // 8
            negA_[:, j] = np.exp(A_log[e, s])
        msk0 = np.full((XD, 1), 1.0 if b == 0 else 0.0, np.float32)
        msk1 = np.full((XD, 1), 1.0 if b == 1 else 0.0, np.float32)
        cw = conv_w[sl]
        conv_d = np.zeros((128, NET * D_CONV * 128), np.float32)
        for k in range(NET):
            for tap in range(D_CONV):
                base = 128 * (D_CONV * k + tap)
                conv_d[np.arange(128), base + np.arange(128)] = cw[128 * k:128 * (k + 1), tap]
        in_maps.append({
            'xb': f32(x[b]),
            'xq': f32(x[b][TQ * r:TQ * (r + 1)]),
            'w_in_f8': f8(np.concatenate([in_proj_w[sl], in_proj_w[D_INNER:][sl]], 0).T * 256.0),
            'conv_wc': col_fold(conv_w[sl]),
            'conv_d': bf(conv_d),
            'conv_bc': col_fold(conv_b[sl]),
            'w_xp_T': bf(x_proj_w[:, sl].T),
            'w_dt_T': bf(dt_proj_w[sl].T),
            'dt_bnc': col_fold(-dt_proj_b[sl]),
            'negA': negA_,
            'd_c': col_fold(D[sl]) * 8.0,
            'w_out_f8': f8(out_proj_w[:, sl].T * 256.0),
            'w_fc_T': bf(c_fc_w.T),
            'w_pr_T': bf(c_proj_w.T),
            'r01': bf(r01.reshape(16 * 128, 128)),
            'g01': bf(g01.reshape(16 * 128, 128) * 8.0),
            's01n': bf(s01n),
            's01p': bf(s01p),
            'ident_bf': bf(ident),
            'ident_f32': f32(ident),
            'ones_bf': bf(np.ones((128, 1), np.float32)),
            'ones_row_bf': bf(np.ones((1, 128), np.float32)),
            'msk0': msk0,
            'msk1': msk1,
        })
    return in_maps


def kernel(**inputs) -> np.ndarray:
    if 'nc' not in _CACHE:
        _CACHE['nc'] = _build()
    nc = _CACHE['nc']
    in_maps = _prep_inputs(inputs)
    res = run_bass_kernel_spmd(nc, in_maps, core_ids=list(range(8)))
    out = np.zeros((B, T, D_MODEL), np.float32)
    for c in range(8):
        b, r = c // 4, c % 4
        out[b, TQ * r:TQ * (r + 1), :] = res.results[c]['out']
    return out

